# revision 1
# baseline (speedup 1.0000x reference)
"""Trainium2 Bass kernel for nn_ContRepDecoder (8-core SPMD, data-parallel over query points).

Strategy:
- Host builds a (32768, 1024) bf16 table: row v = concat over the 8 cube corners of the
  96 context features (zero-padded to 128) at voxel v+offset.
- Each core gathers its points' rows with dma_gather(transpose=True) -> feature-major
  [128, 8, M] tiles (corner-blocked), no on-chip transposes.
- Coordinate features (q, cc, rel_norm, fourier enc) are assembled into one f32
  "coordinate stack" SST [128, M] via tiny-K matmuls into PSUM + ACT Sin + range reduce.
- MLP: per corner 2 matmuls per L0 (gathered h + per-corner masked coord lhsT),
  silu on 2-corner psum pairs, residuals on DVE, trilinear combine via outer-product
  weight psum + pre-scaled post matmuls accumulating into one [45, M] psum.
- Offset-1 coordinate features are affine in offset-0 ones; the differences are folded
  into per-corner bias rows (host weights) + device-computed delta terms from the
  global rel minima (computed once per core over the FULL query set).
"""
import sys

for _p in ("/opt/trn_rl_repo",):
    if _p not in sys.path:
        sys.path.insert(0, _p)

import numpy as np
import ml_dtypes

import concourse.bass as bass
import concourse.mybir as mybir
import concourse.tile as tile

P = 128
C = 96
NFRQ = 8
GRID = 32
QE = 48
NPTS = QE ** 3
NCORE = 8
NSH = NPTS // NCORE
M = 512
EPS = 1e-7
PI = float(np.pi)
F32 = mybir.dt.float32
BF16 = mybir.dt.bfloat16
I16 = mybir.dt.int16
AF = mybir.ActivationFunctionType
ALU = mybir.AluOpType

# SST partition layout
NSIN, NCOS, NQ, NCC, NRN, NONE = 0, 48, 96, 99, 102, 105
BF = ml_dtypes.bfloat16


def corner_off(c):
    return (c >> 2 & 1, c >> 1 & 1, c & 1)


def blob_layout(wfull):
    """(name, rows, cols) entries packed along the free dim of one [128, W] f32 blob."""
    ents = [("wcoord", 128, 16 * C), ("hbm", C, 16), ("wdsm", 3, 16 * C),
            ("bl", C, 4), ("postb", 45, 1), ("emr", 3, P), ("emq", 3, P),
            ("emn", 3, P), ("emd", 3, P), ("e105", 1, P), ("offc", P, 1),
            ("negpic", C, 1), ("coeffidx", 48, 16), ("sel6", 3, 48),
            ("ec8", 8, 8 * C), ("ident", P, P), ("qw", 48, wfull)]
    off, lay = 0, {}
    for nm, r, cc in ents:
        lay[nm] = (off, r, cc)
        off += cc
    return lay, off


BLAY_BF = [("wh0", P, C), ("wh1", C, C), ("w01", C, C), ("w02", C, C),
           ("w11", C, C), ("w12", C, C), ("wpost", C, 45)]


def _enc_row(d, part, k):
    # row of enc feature (dim d, part 0=sin 1=cos, freq k) within the 60-row coord block
    return 12 + d * 16 + part * 8 + k


def host_prep(inputs):
    cse = np.asarray(inputs["context_spatial_extent"], np.float32)
    origin = cse[0, :, 0, 0, 0].copy()
    vox = np.abs(cse[0, :, 1, 1, 1] - cse[0, :, 0, 0, 0])
    qvs = np.asarray(inputs["query_vox_size"], np.float32)[0]
    qflat = np.asarray(inputs["query_coord"], np.float32)[0].reshape(3, NPTS)

    ctx_t = np.moveaxis(np.asarray(inputs["context_v"], np.float32)[0], 0, -1)
    tmp = np.zeros((33, 33, 33, C), np.float32)
    tmp[:32, :32, :32] = ctx_t
    table = np.zeros((GRID ** 3, NCORE * P), np.float32)
    for c in range(8):
        i, j, k = corner_off(c)
        table[:, c * P:c * P + C] = tmp[i:i + 32, j:j + 32, k:k + 32, :].reshape(GRID ** 3, C)
    table = table.astype(BF)

    wfull = NPTS // 16
    qw = qflat.reshape(3, wfull, 16).transpose(0, 2, 1).reshape(48, wfull).copy()

    freqs = (5.0 ** (np.arange(NFRQ) / NFRQ)).astype(np.float32)
    ws = {k: np.asarray(v, np.float32) for k, v in inputs.items() if k.startswith(("blk", "post"))}

    def coord_lhsT(w0, b0):
        wc = w0[C:, :]
        out = np.zeros((8, P, C), np.float32)
        hostbias = np.zeros((8, C), np.float32)
        wds = np.zeros((8, 3, C), np.float32)
        for c in range(8):
            off = corner_off(c)
            hb = b0 + qvs @ wc[0:3]
            for d in range(3):
                o = off[d]
                for k in range(NFRQ):
                    out[c, NSIN + (2 * d + o) * 8 + k, :] = wc[_enc_row(d, 0, k)]
                    out[c, NCOS + (2 * d + o) * 8 + k, :] = wc[_enc_row(d, 1, k)]
                out[c, NQ + d, :] = wc[6 + d]
                out[c, NCC + d, :] = wc[3 + d]
                out[c, NRN + d, :] = wc[9 + d]
                wds[c, d, :] = o * wc[9 + d]
                hb = hb + (off[d] * float(vox[d])) * wc[3 + d]
            hostbias[c] = hb
        return out, hostbias, wds

    c0, hb0, wd0 = coord_lhsT(ws["blk0_w0"], ws["blk0_b0"])
    c1, hb1, wd1 = coord_lhsT(ws["blk1_w0"], ws["blk1_b0"])
    wcoord = np.concatenate([c0, c1], 0).transpose(1, 0, 2).reshape(P, 16 * C).copy()
    hbm = np.concatenate([hb0, hb1], 0).T.copy()
    wdsm = np.concatenate([wd0, wd1], 0).transpose(1, 0, 2).reshape(3, 16 * C).copy()

    wh0 = np.zeros((P, C), np.float32)
    wh0[:C] = ws["blk0_w0"][:C]

    emr = np.zeros((3, P), np.float32)
    emq = np.zeros((3, P), np.float32)
    emn = np.zeros((3, P), np.float32)
    emd = np.zeros((3, P), np.float32)
    for d in range(3):
        for o in (0, 1):
            for k in range(NFRQ):
                w = 2 * PI * freqs[k]
                emr[d, NSIN + (2 * d + o) * 8 + k] = w
                emr[d, NCOS + (2 * d + o) * 8 + k] = w
                if o == 1:
                    emd[d, NSIN + (2 * d + 1) * 8 + k] = w
                    emd[d, NCOS + (2 * d + 1) * 8 + k] = w
        emr[d, NRN + d] = 1.0
        emq[d, NQ + d] = 1.0
        emn[d, NCC + d] = float(vox[d])
    e105 = np.zeros((1, P), np.float32)
    e105[0, NONE] = 1.0
    e105[0, NCOS:NCOS + 48] = 0.5 * PI   # cos = sin(x + pi/2), folded into psum consts

    offc = np.zeros((P, 1), np.float32)
    offc[NSIN:NSIN + 48] = PI
    offc[NCOS:NCOS + 48] = 1.5 * PI
    negpic = np.full((C, 1), -PI, np.float32)

    coeffidx = np.zeros((48, 16), np.float32)
    for ch, w in enumerate((GRID * GRID, GRID, 1)):
        for j in range(16):
            coeffidx[ch * 16 + j, j] = w
    ident = np.eye(P, dtype=np.float32)

    # trilinear factor selection [3, 48]: 6 blocks of [3,8]
    # F1 (x-offset i) uses t-row 2 (tx), F2 (j) row 1, F3 (k) row 0
    sel6 = np.zeros((3, 48), np.float32)
    for c in range(8):
        i, j, k = corner_off(c)
        sel6[2, 0 * 8 + c] = i       # F1 from tw
        sel6[2, 1 * 8 + c] = 1 - i   # F1 from om
        sel6[1, 2 * 8 + c] = j
        sel6[1, 3 * 8 + c] = 1 - j
        sel6[0, 4 * 8 + c] = k
        sel6[0, 5 * 8 + c] = 1 - k
    ec8 = np.zeros((8, 8 * C), np.float32)
    for c in range(8):
        ec8[c, c * C:(c + 1) * C] = 1.0

    # isotropic geometry -> immediate scalars
    assert np.allclose(vox, vox[0]) and np.allclose(origin, origin[0])
    geo = dict(orig=float(origin[0]), ivox=float(np.float32(1.0) / vox[0]),
               vox=float(vox[0]), clamp=float(-vox[0] / 2 + np.float32(EPS)),
               i15=float(np.float32(1.0) / (np.float32(1.5) * vox[0])))

    return dict(
        table=table, qw=qw, qflat=qflat,
        wcoord=wcoord, hbm=hbm, wdsm=wdsm,
        wh0=wh0.astype(BF), wh1=ws["blk1_w0"][:C].astype(BF),
        w01=ws["blk0_w1"].astype(BF), w02=ws["blk0_w2"].astype(BF),
        w11=ws["blk1_w1"].astype(BF), w12=ws["blk1_w2"].astype(BF),
        wpost=ws["post_w"].astype(BF),
        bl=np.stack([ws["blk0_b1"], ws["blk0_b2"], ws["blk1_b1"], ws["blk1_b2"]], 1).copy(),
        postb=ws["post_b"][:, None].copy(),
        emr=emr, emq=emq, emn=emn, emd=emd, e105=e105, offc=offc, negpic=negpic,
        coeffidx=coeffidx, ident=ident, sel6=sel6, ec8=ec8, geo=geo,
        origin=origin, vox=vox,
    )


def build(nsh=NSH, nfull=NPTS, act_silu=True, geo=None, legalize=True):
    geo = geo or dict(orig=0.0, ivox=0.5, vox=2.0, clamp=float(-1.0 + np.float32(EPS)), i15=float(np.float32(1.0) / np.float32(3.0)))
    G_OR, G_IV, G_VX, G_CL, G_I15 = geo["orig"], geo["ivox"], geo["vox"], geo["clamp"], geo["i15"]
    MAGIC = float(2.0 ** 23)
    INV2PI = float(np.float32(1.0) / np.float32(2 * PI))
    NEG2PI = float(-2 * PI)
    wfull = nfull // 16
    wsh = nsh // 16
    nt = nsh // M
    act_fn = AF.Silu if act_silu else AF.Tanh

    lay, ncols = blob_layout(wfull)
    nbcols = sum(cc for _, _, cc in BLAY_BF)
    nc = bass.Bass()
    dp = nc.declare_dram_parameter
    tableD = dp("table", [GRID ** 3, NCORE * P], BF16, isOutput=False)
    qcD = dp("qc", [3, nsh], F32, isOutput=False)
    qwshD = dp("qwsh", [48, wsh], F32, isOutput=False)
    cblobD = dp("cblob", [P, ncols], F32, isOutput=False)
    bblobD = dp("bblob", [P, nbcols], BF16, isOutput=False)
    outD = dp("out", [45, nsh], F32, isOutput=True)

    TS, TT = nc.vector.tensor_scalar, nc.vector.tensor_tensor
    MMX = nc.tensor.matmul
    ACT = nc.scalar.activation

    from contextlib import ExitStack
    with tile.TileContext(nc) as tc, ExitStack() as es:
        cp = es.enter_context(tc.tile_pool(name="const", bufs=1))
        p1 = es.enter_context(tc.tile_pool(name="ph1", bufs=1))
        wp = es.enter_context(tc.tile_pool(name="work", bufs=2))
        pp = es.enter_context(tc.tile_pool(name="ps", bufs=1, space="PSUM"))

        from concourse import library_config
        nc.gpsimd.load_library(library_config.mlp)

        cbt = cp.tile([P, ncols], F32, tag="cblob")
        nc.sync.dma_start(cbt[:, :], cblobD[:, :])
        bb = cp.tile([P, nbcols], BF16, tag="bblob")
        nc.sync.dma_start(bb[:, :], bblobD[:, :])

        def cv(nm):
            o, r, cc = lay[nm]
            return cbt[0:r, o:o + cc]
        bf_off = {}
        _o = 0
        for nm, r, cc in BLAY_BF:
            bf_off[nm] = (_o, r, cc)
            _o += cc

        def bv(nm):
            o, r, cc = bf_off[nm]
            return bb[0:r, o:o + cc]
        wcoord = cv("wcoord")
        hbm = cv("hbm")
        wds = cv("wdsm")
        bl = cv("bl")
        postb = cv("postb")
        emr = cv("emr")
        emq = cv("emq")
        emn = cv("emn")
        emd = cv("emd")
        e105 = cv("e105")
        offc = cv("offc")
        negpic = cv("negpic")
        coeff = cv("coeffidx")
        sel6 = cv("sel6")
        ec8 = cv("ec8")
        ident = cv("ident")
        wh0 = bv("wh0")
        wh1 = bv("wh1")
        wl = {k: bv(k) for k in ("w01", "w02", "w11", "w12")}
        wpost = bv("wpost")
        ones1 = cp.tile([1, M], F32, tag="ones1")
        nc.vector.memset(ones1[:, :], 1.0)
        ones11 = cp.tile([1, 1], F32, tag="ones11")
        nc.vector.memset(ones11[:, :], 1.0)
        ones96 = cp.tile([1, C], F32, tag="ones96")
        nc.vector.memset(ones96[:, :], 1.0)
        zc = cp.tile([C, 1], F32, tag="zc")
        nc.vector.memset(zc[:, :], 0.0)

        def nearest_chunk(srctile, c0, wc, tagp="p1"):
            """Compute clipped nearest (f32) for a chunk of a resident wrapped tile."""
            qch = srctile[:, c0:c0 + wc]
            y = p1.tile([48, CH1], F32, tag=tagp + "a", name="nck_y")
            TS(out=y[:, :wc], in0=qch, scalar1=G_OR, scalar2=G_IV,
               op0=ALU.subtract, op1=ALU.mult)
            n = p1.tile([48, CH1], F32, tag=tagp + "c", name="nck_n")
            TS(out=n[:, :wc], in0=y[:, :wc], scalar1=MAGIC, scalar2=MAGIC,
               op0=ALU.add, op1=ALU.subtract)
            TS(out=n[:, :wc], in0=n[:, :wc], scalar1=0.0, scalar2=float(GRID - 2),
               op0=ALU.max, op1=ALU.min)
            return qch, n

        CH1 = 1728
        qwsh = cp.tile([48, wsh], F32, tag="qwsh")
        nc.sync.dma_start(qwsh[:, :], qwshD[:, :])

        # ===== PHASE 2: gather indices for this core's shard ====
        idx128 = cp.tile([P, wsh], I16, tag="idx128")
        ch = 432
        for h in range(0, wsh, ch):
            wc = min(ch, wsh - h)
            _, nsw = nearest_chunk(qwsh, h, wc, tagp="p2")
            ix_ps = pp.tile([128, M], F32, tag="scr", space="PSUM")
            MMX(ix_ps[0:16, 0:wc], coeff[:, :], nsw[:, :wc], start=True, stop=True)
            nc.vector.tensor_copy(idx128[0:16, h:h + wc], ix_ps[0:16, 0:wc])
        for rep in (16, 32, 64):
            nc.gpsimd.dma_start(out=idx128[rep:2 * rep, :], in_=idx128[0:rep, :])

        # ===== PHASE 1: global rel minima over the FULL query set (wrapped layout) ====
        CH1 = 1728
        nch1 = (wfull + CH1 - 1) // CH1
        qwall = cv("qw")
        qwsh = cp.tile([48, wsh], F32, tag="qwsh")
        nc.sync.dma_start(qwsh[:, :], qwshD[:, :])


        m2ch = p1.tile([48, 2 * nch1], F32, tag="m2ch")
        for chi in range(nch1):
            c0 = chi * CH1
            wc = min(CH1, wfull - c0)
            qch, n = nearest_chunk(qwall, c0, wc)
            cw = p1.tile([48, CH1], F32, tag="p1a")
            TS(out=cw[:, :wc], in0=n[:, :wc], scalar1=G_VX, scalar2=G_OR,
               op0=ALU.mult, op1=ALU.add)
            dw = p1.tile([48, CH1], F32, tag="p1b")
            TT(out=dw[:, :wc], in0=cw[:, :wc], in1=qch, op=ALU.subtract)
            r0 = p1.tile([48, CH1], F32, tag="p1a")
            TS(out=r0[:, :wc], in0=dw[:, :wc], scalar1=G_CL, scalar2=None, op0=ALU.max)
            nc.vector.tensor_reduce(out=m2ch[:, chi:chi + 1], in_=r0[:, :wc],
                                    axis=mybir.AxisListType.X, op=ALU.min)
            r1 = p1.tile([48, CH1], F32, tag="p1c")
            TS(out=r1[:, :wc], in0=dw[:, :wc], scalar1=G_VX, scalar2=G_CL,
               op0=ALU.add, op1=ALU.max)
            nc.vector.tensor_reduce(out=m2ch[:, nch1 + chi:nch1 + chi + 1], in_=r1[:, :wc],
                                    axis=mybir.AxisListType.X, op=ALU.min)
        m2 = p1.tile([48, 2], F32, tag="m2")
        nc.vector.tensor_reduce(out=m2[:, 0:1], in_=m2ch[:, 0:nch1],
                                axis=mybir.AxisListType.X, op=ALU.min)
        nc.vector.tensor_reduce(out=m2[:, 1:2], in_=m2ch[:, nch1:2 * nch1],
                                axis=mybir.AxisListType.X, op=ALU.min)
        mt_ps = pp.tile([128, M], F32, tag="scr", space="PSUM")
        nc.tensor.transpose(out=mt_ps[0:2, 0:48], in_=m2[:, :], identity=ident[0:48, 0:48])
        mts = p1.tile([2, 48], F32, tag="mts")
        ACT(out=mts[:, :], in_=mt_ps[0:2, 0:48], func=AF.Copy)
        mn6 = p1.tile([2, 3], F32, tag="mn6")
        for d in range(3):
            nc.vector.tensor_reduce(out=mn6[:, d:d + 1], in_=mts[:, 16 * d:16 * (d + 1)],
                                    axis=mybir.AxisListType.X, op=ALU.min)
        mnt_ps = pp.tile([128, M], F32, tag="scr", space="PSUM")
        nc.tensor.transpose(out=mnt_ps[0:3, 0:2], in_=mn6[:, :], identity=ident[0:2, 0:2])
        mns = p1.tile([3, 2], F32, tag="mns")
        ACT(out=mns[:, :], in_=mnt_ps[0:3, 0:2], func=AF.Copy)
        # delta_d = (vox + min0 - min1) / (1.5 vox); negm0c = -min0/(1.5 vox)
        dcol = p1.tile([3, 1], F32, tag="dcol")
        TT(out=dcol[:, :], in0=mns[:, 0:1], in1=mns[:, 1:2], op=ALU.subtract)
        TS(out=dcol[:, :], in0=dcol[:, :], scalar1=G_VX, scalar2=G_I15,
           op0=ALU.add, op1=ALU.mult)
        negm0c = p1.tile([3, 1], F32, tag="negm0c")
        TS(out=negm0c[:, :], in0=mns[:, 0:1], scalar1=G_I15, scalar2=-1.0,
           op0=ALU.mult, op1=ALU.mult)
        # offrow [1,128]: delta angle offsets + ones row marker
        or_ps = pp.tile([128, M], F32, tag="scr", space="PSUM")
        MMX(or_ps[0:P, 0:1], emd[:, :], dcol[:, :], start=True, stop=False)
        MMX(or_ps[0:P, 0:1], e105[:, :], ones11[:, :], start=False, stop=True)
        orS = p1.tile([P, 1], F32, tag="orS")
        ACT(out=orS[:, :], in_=or_ps[0:P, 0:1], func=AF.Copy)
        ort_ps = pp.tile([128, M], F32, tag="scr", space="PSUM")
        nc.tensor.transpose(out=ort_ps[0:1, 0:P], in_=orS[:, :], identity=ident[:, :])
        offrow = p1.tile([1, P], F32, tag="offrow")
        ACT(out=offrow[:, :], in_=ort_ps[0:1, 0:P], func=AF.Copy)
        # per-(block,corner) bias rows -> wcoord row NONE via transpose + DMAs
        db_ps = pp.tile([128, M], F32, tag="scr", space="PSUM")
        for cb in range(16):
            MMX(db_ps[0:C, cb:cb + 1], wds[:, cb * C:(cb + 1) * C], dcol[:, :],
                start=True, stop=True)
        biasT = p1.tile([C, 16], F32, tag="biasT")
        TT(out=biasT[:, :], in0=db_ps[0:C, 0:16], in1=hbm[:, :], op=ALU.add)
        bt_ps = pp.tile([128, M], F32, tag="scr", space="PSUM")
        nc.tensor.transpose(out=bt_ps[0:16, 0:C], in_=biasT[:, :], identity=ident[0:C, 0:C])
        biasTT = p1.tile([16, C], F32, tag="biasTT")
        ACT(out=biasTT[:, :], in_=bt_ps[0:16, 0:C], func=AF.Copy)
        for cb in range(16):
            nc.gpsimd.dma_start(out=wcoord[NONE:NONE + 1, cb * C:(cb + 1) * C],
                                in_=biasTT[cb:cb + 1, :])

        # ===== PHASE 3: main loop ====
        qcv = qcD[:, :]
        for t in range(nt):
            cs = t * M
            GT = wp.tile([P, NCORE, M], BF16, tag="gt")
            nc.gpsimd.dma_gather(
                out_ap=GT[:, :, :], in_ap=tableD[:, :],
                idxs_ap=idx128[:, t * (M // 16):(t + 1) * (M // 16)],
                num_idxs=M, num_idxs_reg=M, elem_size=NCORE * P,
                transpose=True, queue_num=0,
            )
            qq = wp.tile([3, M], F32, tag="qq")
            nc.sync.dma_start(qq[:, :], qcv[:, cs:cs + M])
            yq = wp.tile([3, M], F32, tag="qsA")
            TS(out=yq[:, :], in0=qq[:, :], scalar1=G_OR,
               scalar2=G_IV, op0=ALU.subtract, op1=ALU.mult)
            n3q = wp.tile([3, M], F32, tag="n3q")
            TS(out=n3q[:, :], in0=yq[:, :], scalar1=MAGIC, scalar2=MAGIC,
               op0=ALU.add, op1=ALU.subtract)
            TS(out=n3q[:, :], in0=n3q[:, :], scalar1=0.0, scalar2=float(GRID - 2),
               op0=ALU.max, op1=ALU.min)
            ccq = wp.tile([3, M], F32, tag="qsA")
            TS(out=ccq[:, :], in0=n3q[:, :], scalar1=G_VX,
               scalar2=G_OR, op0=ALU.mult, op1=ALU.add)
            r0q = wp.tile([3, M], F32, tag="qsB")
            TT(out=r0q[:, :], in0=ccq[:, :], in1=qq[:, :], op=ALU.subtract)
            TS(out=r0q[:, :], in0=r0q[:, :], scalar1=G_CL,
               scalar2=None, op0=ALU.max)
            rnq = wp.tile([3, M], F32, tag="rnq")
            TS(out=rnq[:, :], in0=r0q[:, :], scalar1=G_I15,
               scalar2=negm0c[:, 0:1], op0=ALU.mult, op1=ALU.add)
            twq = wp.tile([3, M], F32, tag="twq")
            TS(out=twq[:, :], in0=r0q[:, :], scalar1=-0.5,
               scalar2=-(1.0 - EPS) / 2, op0=ALU.add, op1=ALU.max)
            TS(out=twq[:, :], in0=twq[:, :], scalar1=(1.0 - EPS) / 2,
               scalar2=0.5, op0=ALU.min, op1=ALU.add)
            omq = wp.tile([3, M], F32, tag="omq")
            TS(out=omq[:, :], in0=twq[:, :], scalar1=-1.0, scalar2=1.0,
               op0=ALU.mult, op1=ALU.add)

            ang = pp.tile([P, M], F32, tag="ang", space="PSUM")
            MMX(ang[:, :], emr[:, :], rnq[:, :], start=True, stop=False)
            MMX(ang[:, :], emq[:, :], qq[:, :], start=False, stop=False)
            MMX(ang[:, :], emn[:, :], n3q[:, :], start=False, stop=False)
            MMX(ang[:, :], offrow[:, :], ones1[:, :], start=False, stop=True)
            sst = wp.tile([P, M], F32, tag="sst")
            rr = wp.tile([P, M], F32, tag="rr")
            TS(out=rr[0:C, :], in0=ang[0:C, :], scalar1=INV2PI, scalar2=MAGIC,
               op0=ALU.mult, op1=ALU.add)
            TS(out=rr[0:C, :], in0=rr[0:C, :], scalar1=MAGIC, scalar2=NEG2PI,
               op0=ALU.subtract, op1=ALU.mult)
            TT(out=sst[0:C, :], in0=ang[0:C, :], in1=rr[0:C, :], op=ALU.add)
            ACT(out=sst[0:C, :], in_=sst[0:C, :], func=AF.Sin, bias=0.0)
            ACT(out=sst[C:P, :], in_=ang[C:P, :], func=AF.Copy)

            # trilinear factors: F1/F2/F3 [8, M] psums via selection matmuls, then products
            fps = []
            for fi, tg in enumerate(("scr", "wcp", "ang")):
                f_ps = pp.tile([P, M], F32, tag=tg, space="PSUM",
                               name=f"f{fi}_ps")
                MMX(f_ps[0:8, :], sel6[:, fi * 16:fi * 16 + 8], twq[:, :],
                    start=True, stop=False)
                MMX(f_ps[0:8, :], sel6[:, fi * 16 + 8:fi * 16 + 16], omq[:, :],
                    start=False, stop=True)
                fps.append(f_ps)
            f2s = wp.tile([8, M], F32, tag="f2s")
            nc.vector.tensor_copy(f2s[:, :], fps[1][0:8, :])
            w12t = wp.tile([8, M], F32, tag="w12t")
            TT(out=w12t[:, :], in0=fps[0][0:8, :], in1=f2s[:, :], op=ALU.mult)
            wct8 = wp.tile([8, M], F32, tag="wct8")
            TT(out=wct8[:, :], in0=w12t[:, :], in1=fps[2][0:8, :], op=ALU.mult)

            pout = pp.tile([45, M], F32, tag="pout", space="PSUM")
            for cpair in range(4):
                cA = 2 * cpair
                lp0 = pp.tile([C, 2 * M], F32, tag="lp0", space="PSUM")
                for ci in range(2):
                    sl = slice(ci * M, (ci + 1) * M)
                    MMX(lp0[:, sl], wh0[:, :], GT[:, cA + ci, :], start=True, stop=False)
                for ci in range(2):
                    cc_ = cA + ci
                    sl = slice(ci * M, (ci + 1) * M)
                    MMX(lp0[:, sl], wcoord[:, cc_ * C:(cc_ + 1) * C], sst[:, :],
                        start=False, stop=True)
                t1p = wp.tile([C, 2 * M], BF16, tag="t1p", bufs=3)
                ACT(out=t1p[:, :], in_=lp0[:, :], func=act_fn, bias=zc[:, 0:1])
                lp1 = pp.tile([C, 2 * M], F32, tag="lp1", space="PSUM")
                for ci in range(2):
                    sl = slice(ci * M, (ci + 1) * M)
                    MMX(lp1[:, sl], wl["w01"][:, :], t1p[:, sl], start=True, stop=True)
                t2p = wp.tile([C, 2 * M], BF16, tag="t2p", bufs=3)
                ACT(out=t2p[:, :], in_=lp1[:, :], func=act_fn, bias=bl[:, 0:1])
                lp2 = pp.tile([C, 2 * M], F32, tag="lp0", space="PSUM")
                for ci in range(2):
                    sl = slice(ci * M, (ci + 1) * M)
                    MMX(lp2[:, sl], wl["w02"][:, :], t2p[:, sl], start=True, stop=True)
                t3p = wp.tile([C, 2 * M], BF16, tag="t3p", bufs=3)
                ACT(out=t3p[:, :], in_=lp2[:, :], func=act_fn, bias=bl[:, 1:2])
                h1p = wp.tile([C, 2 * M], BF16, tag="h1p")
                TT(out=h1p[:, :], in0=GT[0:C, cA:cA + 2, :], in1=t3p[:, :], op=ALU.add)

                lp3 = pp.tile([C, 2 * M], F32, tag="lp1", space="PSUM")
                for ci in range(2):
                    sl = slice(ci * M, (ci + 1) * M)
                    MMX(lp3[:, sl], wh1[:, :], h1p[:, sl], start=True, stop=False)
                for ci in range(2):
                    cc_ = cA + ci
                    sl = slice(ci * M, (ci + 1) * M)
                    MMX(lp3[:, sl], wcoord[:, (8 + cc_) * C:(9 + cc_) * C], sst[:, :],
                        start=False, stop=True)
                u1p = wp.tile([C, 2 * M], BF16, tag="t1p", bufs=3, name="u1p")
                ACT(out=u1p[:, :], in_=lp3[:, :], func=act_fn, bias=zc[:, 0:1])
                lp4 = pp.tile([C, 2 * M], F32, tag="lp0", space="PSUM")
                for ci in range(2):
                    sl = slice(ci * M, (ci + 1) * M)
                    MMX(lp4[:, sl], wl["w11"][:, :], u1p[:, sl], start=True, stop=True)
                u2p = wp.tile([C, 2 * M], BF16, tag="t2p", bufs=3, name="u2p")
                ACT(out=u2p[:, :], in_=lp4[:, :], func=act_fn, bias=bl[:, 2:3])
                lp5 = pp.tile([C, 2 * M], F32, tag="lp1", space="PSUM")
                for ci in range(2):
                    sl = slice(ci * M, (ci + 1) * M)
                    MMX(lp5[:, sl], wl["w12"][:, :], u2p[:, sl], start=True, stop=True)
                u3p = wp.tile([C, 2 * M], BF16, tag="t3p", bufs=3, name="u3p")
                ACT(out=u3p[:, :], in_=lp5[:, :], func=act_fn, bias=bl[:, 3:4])
                h2p = wp.tile([C, 2 * M], BF16, tag="h2p")
                TT(out=h2p[:, :], in0=h1p[:, :], in1=u3p[:, :], op=ALU.add)

                h2ss = []
                for ci in range(2):
                    cc_ = cA + ci
                    sl = slice(ci * M, (ci + 1) * M)
                    wcp = pp.tile([P, M], F32, tag="wcp", space="PSUM")
                    MMX(wcp[0:C, :], ec8[:, cc_ * C:(cc_ + 1) * C], wct8[:, :],
                        start=True, stop=True)
                    h2s = wp.tile([C, M], BF16, tag=f"h2s{ci}", name=f"h2s{ci}")
                    TT(out=h2s[:, :], in0=h2p[:, sl], in1=wcp[0:C, :], op=ALU.mult)
                    h2ss.append(h2s)
                for ci in range(2):
                    cc_ = cA + ci
                    MMX(pout[:, :], wpost[:, :], h2ss[ci][:, :], start=(cc_ == 0),
                        stop=(cc_ == 7))
            osb = wp.tile([45, M], F32, tag="osb")
            ACT(out=osb[:, :], in_=pout[:, :], func=AF.Identity, bias=postb[:, 0:1])
            nc.sync.dma_start(outD[:, cs:cs + M], osb[:, :])

    from concourse.library_overlay import lower_extended_insts
    lower_extended_insts(nc)
    if legalize:
        _legalize_waits(nc)
    return nc


def _wait_limit(ins):
    return 1


def _legalize_waits(nc):
    """The walrus codegen allows only one sync-wait command per normal
    instruction. Split excess waits onto dedicated same-engine NOPs
    inserted immediately before the instruction (program position is
    unchanged, so dependency semantics are identical)."""
    import bass_rust as _br
    nid = 0
    for fn in nc.m.functions:
        for bb in fn.blocks:
            il = bb.instructions
            i = 0
            while i < len(il):
                ins = il[i]
                si = ins.sync_info
                lim = _wait_limit(ins)
                if si is not None and si.on_wait and len(si.on_wait) > lim:
                    ow = list(si.on_wait)
                    keep, excess = ow[-lim:], ow[:-lim]
                    for w in excess:
                        nid += 1
                        nop = mybir.InstNoOp(name=f"legwait-{nid}", ins=[], outs=[])
                        nop.engine = ins.engine
                        nop.sync_info = _br.SyncInfo(on_wait=[w], on_update=[])
                        il.insert(i, nop)
                        i += 1
                    si.on_wait = keep
                i += 1


def make_in_maps(inputs, cst=None, nsh=NSH, ncore=NCORE):
    shared, percore = make_split_maps(inputs, cst, nsh, ncore)
    return [dict(shared, **pc) for pc in percore]


def make_split_maps(inputs, cst=None, nsh=NSH, ncore=NCORE):
    cst = cst or host_prep(inputs)
    qflat = cst["qflat"]
    wsh = nsh // 16
    wfull = cst["qw"].shape[1]
    lay, ncols = blob_layout(wfull)
    cblob = np.zeros((P, ncols), np.float32)
    for nm, (o, r, cc) in lay.items():
        arr = cst[nm]
        assert arr.shape == (r, cc), (nm, arr.shape, (r, cc))
        cblob[0:r, o:o + cc] = arr
    nb = sum(cc for _, _, cc in BLAY_BF)
    bblob = np.zeros((P, nb), BF)
    _o = 0
    for nm, r, cc in BLAY_BF:
        bblob[0:r, _o:_o + cc] = cst[nm]
        _o += cc
    shared = dict(table=np.ascontiguousarray(cst["table"]),
                  bblob=np.ascontiguousarray(bblob),
                  cblob=np.ascontiguousarray(cblob))
    percore = []
    for core in range(ncore):
        sl = qflat[:, core * nsh:(core + 1) * nsh]
        percore.append(dict(
            qc=np.ascontiguousarray(sl),
            qwsh=np.ascontiguousarray(
                sl.reshape(3, wsh, 16).transpose(0, 2, 1).reshape(48, wsh)),
        ))
    return shared, percore


_CACHE = {}


def get_runner(nc, ncore=NCORE):
    """Compile an SPMD runner: shared inputs replicated (one transfer),
    per-core inputs sharded along axis 0."""
    import concourse.bass2jax as b2j
    import jax
    from jax.sharding import Mesh, PartitionSpec, NamedSharding
    from jax.experimental.shard_map import shard_map

    b2j.install_neuronx_cc_hook()
    partition_name = nc.partition_id_tensor.name if nc.partition_id_tensor else None
    in_names, out_names, out_avals, zero_outs = [], [], [], []
    for alloc in nc.m.functions[0].allocations:
        if not isinstance(alloc, mybir.MemoryLocationSet):
            continue
        name = alloc.memorylocations[0].name
        if alloc.kind == "ExternalInput":
            if name != partition_name:
                in_names.append(name)
        elif alloc.kind == "ExternalOutput":
            out_names.append(name)
            shape = tuple(alloc.tensor_shape)
            dtype = mybir.dt.np(alloc.dtype)
            out_avals.append(jax.core.ShapedArray(shape, dtype))
            zero_outs.append(np.zeros(shape, dtype))
    n_params = len(in_names)
    all_names = in_names + out_names
    if partition_name is not None:
        all_names.append(partition_name)

    def _body(*args):
        operands = list(args)
        if partition_name is not None:
            operands.append(b2j.partition_id_tensor())
        outs = b2j._bass_exec_p.bind(
            *operands, out_avals=tuple(out_avals), in_names=tuple(all_names),
            out_names=tuple(out_names), lowering_input_output_aliases=(),
            sim_require_finite=True, sim_require_nnan=True, nc=nc)
        return tuple(outs)

    devices = jax.devices()[:ncore]
    mesh = Mesh(np.asarray(devices), ("core",))

    def runner(shared, percore):
        specs, arrs = [], []
        for nm in in_names:
            if nm in shared:
                specs.append(PartitionSpec())
                arrs.append(shared[nm])
            else:
                specs.append(PartitionSpec("core"))
                arrs.append(np.concatenate([pc[nm] for pc in percore], axis=0))
        for z in zero_outs:
            specs.append(PartitionSpec("core"))
            arrs.append(np.concatenate([z] * ncore, axis=0))
        fn = jax.jit(shard_map(_body, mesh=mesh, in_specs=tuple(specs),
                               out_specs=(PartitionSpec("core"),) * len(out_names),
                               check_rep=False), keep_unused=True)
        dev = [jax.device_put(a, NamedSharding(mesh, s)) for a, s in zip(arrs, specs)]
        outs = fn(*dev)
        jax.block_until_ready(outs)
        return {nm: np.asarray(o) for nm, o in zip(out_names, outs)}, (fn, dev)

    return runner


def kernel(**inputs):
    cst = host_prep(inputs)
    if "nc" not in _CACHE:
        _CACHE["nc"] = build(geo=cst["geo"])
        _CACHE["runner"] = get_runner(_CACHE["nc"])
    shared, percore = make_split_maps(inputs, cst)
    outmap, _ = _CACHE["runner"](shared, percore)
    full_cat = outmap["out"]                     # (8*45, NSH)
    full = np.concatenate([full_cat[c * 45:(c + 1) * 45] for c in range(NCORE)], axis=1)
    return full.reshape(1, 45, QE, QE, QE).astype(np.float32)



# revision 16
# speedup vs baseline: 1.1833x; 1.1833x over previous
"""Trainium2 Bass kernel for nn_ContRepDecoder (8-core SPMD, data-parallel over query points).

Strategy:
- Host builds a (32768, 1024) bf16 table: row v = concat over the 8 cube corners of the
  96 context features (zero-padded to 128) at voxel v+offset.
- Each core gathers its points' rows with dma_gather(transpose=True) -> feature-major
  [128, 8, M] tiles (corner-blocked), no on-chip transposes.
- Coordinate features (q, cc, rel_norm, fourier enc) are assembled into one f32
  "coordinate stack" SST [128, M] via tiny-K matmuls into PSUM + ACT Sin + range reduce.
- MLP: per corner 2 matmuls per L0 (gathered h + per-corner masked coord lhsT),
  silu on 2-corner psum pairs, residuals on DVE, trilinear combine via outer-product
  weight psum + pre-scaled post matmuls accumulating into one [45, M] psum.
- Offset-1 coordinate features are affine in offset-0 ones; the differences are folded
  into per-corner bias rows (host weights) + device-computed delta terms from the
  global rel minima (computed once per core over the FULL query set).
"""
import sys

for _p in ("/opt/trn_rl_repo",):
    if _p not in sys.path:
        sys.path.insert(0, _p)

import numpy as np
import ml_dtypes

import concourse.bass as bass
import concourse.mybir as mybir
import concourse.tile as tile

P = 128
C = 96
NFRQ = 8
GRID = 32
QE = 48
NPTS = QE ** 3
NCORE = 8
NSH = NPTS // NCORE
M = 512
EPS = 1e-7
PI = float(np.pi)
F32 = mybir.dt.float32
BF16 = mybir.dt.bfloat16
I16 = mybir.dt.int16
AF = mybir.ActivationFunctionType
ALU = mybir.AluOpType

# SST partition layout
NSIN, NCOS, NQ, NCC, NRN, NONE = 0, 48, 96, 99, 102, 105
BF = ml_dtypes.bfloat16


def corner_off(c):
    return (c >> 2 & 1, c >> 1 & 1, c & 1)


def blob_layout(wfull):
    """(name, rows, cols) entries packed along the free dim of one [128, W] f32 blob."""
    ents = [("hbm", C, 16), ("wdsm", 3, 16 * C),
            ("bl", C, 4), ("postb", 45, 1), ("emstk", 97, P),
            ("emd", 3, P), ("e105", 1, P),
            ("coeffidx", 48, 16), ("sel6", 35, 24),
            ("ident", P, P), ("qw", 48, wfull)]
    off, lay = 0, {}
    for nm, r, cc in ents:
        lay[nm] = (off, r, cc)
        off += cc
    return lay, off


BLAY_BF = [("wh0", P, C), ("wh1", C, C), ("w01", C, C), ("w02", C, C),
           ("w11", C, C), ("w12", C, C), ("wpost", C, 45),
           ("wcoord", P, 16 * C), ("ec8", 8, 8 * C)]
QOFF = 31.0


def _enc_row(d, part, k):
    # row of enc feature (dim d, part 0=sin 1=cos, freq k) within the 60-row coord block
    return 12 + d * 16 + part * 8 + k


def host_prep(inputs):
    cse = np.asarray(inputs["context_spatial_extent"], np.float32)
    origin = cse[0, :, 0, 0, 0].copy()
    vox = np.abs(cse[0, :, 1, 1, 1] - cse[0, :, 0, 0, 0])
    qvs = np.asarray(inputs["query_vox_size"], np.float32)[0]
    qflat = np.asarray(inputs["query_coord"], np.float32)[0].reshape(3, NPTS)

    ctx_t = np.moveaxis(np.asarray(inputs["context_v"], np.float32)[0], 0, -1)
    tmp = np.zeros((33, 33, 33, C), np.float32)
    tmp[:32, :32, :32] = ctx_t
    table = np.zeros((GRID ** 3, NCORE * P), np.float32)
    for c in range(8):
        i, j, k = corner_off(c)
        table[:, c * P:c * P + C] = tmp[i:i + 32, j:j + 32, k:k + 32, :].reshape(GRID ** 3, C)
    table = table.astype(BF)

    wfull = NPTS // 16
    qw = qflat.reshape(3, wfull, 16).transpose(0, 2, 1).reshape(48, wfull).copy()

    freqs = (5.0 ** (np.arange(NFRQ) / NFRQ)).astype(np.float32)
    ws = {k: np.asarray(v, np.float32) for k, v in inputs.items() if k.startswith(("blk", "post"))}

    def coord_lhsT(w0, b0):
        # sst coord-row basis (bf16-safe): v1 = vox*n - QOFF (exact odd ints),
        # v2 = cc0 - q  (in [-1,1]).  q = v1 - v2 + QOFF ; cc0 = v1 + QOFF.
        wc = w0[C:, :]
        out = np.zeros((8, P, C), np.float32)
        hostbias = np.zeros((8, C), np.float32)
        wds = np.zeros((8, 3, C), np.float32)
        for c in range(8):
            off = corner_off(c)
            hb = b0 + qvs @ wc[0:3]
            for d in range(3):
                o = off[d]
                for k in range(NFRQ):
                    out[c, NSIN + (2 * d + o) * 8 + k, :] = wc[_enc_row(d, 0, k)]
                    out[c, NCOS + (2 * d + o) * 8 + k, :] = wc[_enc_row(d, 1, k)]
                out[c, NQ + d, :] = wc[6 + d] + wc[3 + d]
                out[c, NCC + d, :] = -wc[6 + d]
                out[c, NRN + d, :] = wc[9 + d]
                wds[c, d, :] = o * wc[9 + d]
                hb = hb + (off[d] * float(vox[d])) * wc[3 + d] \
                        + QOFF * (wc[6 + d] + wc[3 + d])
            hostbias[c] = hb
        return out, hostbias, wds

    c0, hb0, wd0 = coord_lhsT(ws["blk0_w0"], ws["blk0_b0"])
    c1, hb1, wd1 = coord_lhsT(ws["blk1_w0"], ws["blk1_b0"])
    wcoord = np.concatenate([c0, c1], 0).transpose(1, 0, 2).reshape(P, 16 * C).copy()
    hbm = np.concatenate([hb0, hb1], 0).T.copy()
    wdsm = np.concatenate([wd0, wd1], 0).transpose(1, 0, 2).reshape(3, 16 * C).copy()

    wh0 = np.zeros((P, C), np.float32)
    wh0[:C] = ws["blk0_w0"][:C]

    # ang matmul lhsT stack [97, P] (32-aligned blocks for DVE partition-start
    # legality): rows 0-2 emr (rhs=rnq), 32-34 emq (rhs=qq), 64-66 emn
    # (rhs=n3q), row 96 = runtime offrow (rhs=ones).
    emr = np.zeros((3, P), np.float32)
    emq = np.zeros((3, P), np.float32)
    emn = np.zeros((3, P), np.float32)
    emd = np.zeros((3, P), np.float32)
    for d in range(3):
        for o in (0, 1):
            for k in range(NFRQ):
                w = 2 * PI * freqs[k]
                emr[d, NSIN + (2 * d + o) * 8 + k] = w
                emr[d, NCOS + (2 * d + o) * 8 + k] = w
                if o == 1:
                    emd[d, NSIN + (2 * d + 1) * 8 + k] = w
                    emd[d, NCOS + (2 * d + 1) * 8 + k] = w
        emr[d, NRN + d] = 1.0
        emq[d, NCC + d] = -1.0                 # v2 = vox*n - q
        emn[d, NCC + d] = float(vox[d])
        emn[d, NQ + d] = float(vox[d])         # v1 = vox*n - QOFF
    e105 = np.zeros((1, P), np.float32)
    e105[0, NONE] = 1.0
    e105[0, NCOS:NCOS + 48] = 0.5 * PI   # cos = sin(x + pi/2), folded into psum consts
    e105[0, NQ:NQ + 3] = -QOFF
    emstk = np.zeros((97, P), np.float32)
    emstk[0:3] = emr
    emstk[32:35] = emq
    emstk[64:67] = emn

    coeffidx = np.zeros((48, 16), np.float32)
    for ch, w in enumerate((GRID * GRID, GRID, 1)):
        for j in range(16):
            coeffidx[ch * 16 + j, j] = w
    ident = np.eye(P, dtype=np.float32)

    # trilinear factor selection [35, 24]: 3 blocks of [35,8]; rows 0:3 pick
    # from twq, rows 32:35 from omq (32-aligned stacked rhs).
    # F1 (x-offset i) uses t-row 2 (tx), F2 (j) row 1, F3 (k) row 0
    sel6 = np.zeros((35, 24), np.float32)
    for c in range(8):
        i, j, k = corner_off(c)
        sel6[2, 0 * 8 + c] = i       # F1 from tw
        sel6[34, 0 * 8 + c] = 1 - i  # F1 from om
        sel6[1, 1 * 8 + c] = j
        sel6[33, 1 * 8 + c] = 1 - j
        sel6[0, 2 * 8 + c] = k
        sel6[32, 2 * 8 + c] = 1 - k
    ec8 = np.zeros((8, 8 * C), np.float32)
    for c in range(8):
        ec8[c, c * C:(c + 1) * C] = 1.0

    # isotropic geometry -> immediate scalars
    assert np.allclose(vox, vox[0]) and np.allclose(origin, origin[0])
    geo = dict(orig=float(origin[0]), ivox=float(np.float32(1.0) / vox[0]),
               vox=float(vox[0]), clamp=float(-vox[0] / 2 + np.float32(EPS)),
               i15=float(np.float32(1.0) / (np.float32(1.5) * vox[0])))

    return dict(
        table=table, qw=qw, qflat=qflat,
        wcoord=wcoord.astype(BF), hbm=hbm, wdsm=wdsm,
        wh0=wh0.astype(BF), wh1=ws["blk1_w0"][:C].astype(BF),
        w01=ws["blk0_w1"].astype(BF), w02=ws["blk0_w2"].astype(BF),
        w11=ws["blk1_w1"].astype(BF), w12=ws["blk1_w2"].astype(BF),
        wpost=ws["post_w"].astype(BF),
        bl=np.stack([ws["blk0_b1"], ws["blk0_b2"], ws["blk1_b1"], ws["blk1_b2"]], 1).copy(),
        postb=ws["post_b"][:, None].copy(),
        emstk=emstk, emd=emd, e105=e105,
        coeffidx=coeffidx, ident=ident, sel6=sel6, ec8=ec8.astype(BF), geo=geo,
        origin=origin, vox=vox,
    )


def build(nsh=NSH, nfull=NPTS, act_silu=True, geo=None, legalize=True):
    geo = geo or dict(orig=0.0, ivox=0.5, vox=2.0, clamp=float(-1.0 + np.float32(EPS)), i15=float(np.float32(1.0) / np.float32(3.0)))
    G_OR, G_IV, G_VX, G_CL, G_I15 = geo["orig"], geo["ivox"], geo["vox"], geo["clamp"], geo["i15"]
    MAGIC = float(2.0 ** 23)
    INV2PI = float(np.float32(1.0) / np.float32(2 * PI))
    NEG2PI = float(-2 * PI)
    wfull = nfull // 16
    wsh = nsh // 16
    nt = nsh // M
    act_fn = AF.Silu if act_silu else AF.Tanh

    lay, ncols = blob_layout(wfull)
    nbcols = sum(cc for _, _, cc in BLAY_BF)
    nc = bass.Bass()
    dp = nc.declare_dram_parameter
    tableD = dp("table", [GRID ** 3, NCORE * P], BF16, isOutput=False)
    qcD = dp("qc", [3, nsh], F32, isOutput=False)
    qwshD = dp("qwsh", [48, wsh], F32, isOutput=False)
    cblobD = dp("cblob", [P, ncols], F32, isOutput=False)
    bblobD = dp("bblob", [P, nbcols], BF16, isOutput=False)
    outD = dp("out", [45, nsh], F32, isOutput=True)

    TS, TT = nc.vector.tensor_scalar, nc.vector.tensor_tensor
    MMX = nc.tensor.matmul
    ACT = nc.scalar.activation

    from contextlib import ExitStack
    with tile.TileContext(nc) as tc, ExitStack() as es:
        cp = es.enter_context(tc.tile_pool(name="const", bufs=1))
        p1 = es.enter_context(tc.tile_pool(name="ph1", bufs=1))
        wp = es.enter_context(tc.tile_pool(name="work", bufs=2))
        pp = es.enter_context(tc.tile_pool(name="ps", bufs=1, space="PSUM"))

        from concourse import library_config
        nc.gpsimd.load_library(library_config.mlp)

        cbt = cp.tile([P, ncols], F32, tag="cblob")
        nc.sync.dma_start(cbt[:, :], cblobD[:, :])
        bb = cp.tile([P, nbcols], BF16, tag="bblob")
        nc.sync.dma_start(bb[:, :], bblobD[:, :])

        def cv(nm):
            o, r, cc = lay[nm]
            return cbt[0:r, o:o + cc]
        bf_off = {}
        _o = 0
        for nm, r, cc in BLAY_BF:
            bf_off[nm] = (_o, r, cc)
            _o += cc

        def bv(nm):
            o, r, cc = bf_off[nm]
            return bb[0:r, o:o + cc]
        wcoord = bv("wcoord")
        hbm = cv("hbm")
        wds = cv("wdsm")
        bl = cv("bl")
        postb = cv("postb")
        emstk = cv("emstk")
        emd = cv("emd")
        e105 = cv("e105")
        coeff = cv("coeffidx")
        sel6 = cv("sel6")
        ec8 = bv("ec8")
        ident = cv("ident")
        wh0 = bv("wh0")
        wh1 = bv("wh1")
        wl = {k: bv(k) for k in ("w01", "w02", "w11", "w12")}
        wpost = bv("wpost")
        ones1 = cp.tile([1, M], F32, tag="ones1")
        nc.vector.memset(ones1[:, :], 1.0)
        ones11 = cp.tile([1, 1], F32, tag="ones11")
        nc.vector.memset(ones11[:, :], 1.0)
        ones96 = cp.tile([1, C], F32, tag="ones96")
        nc.vector.memset(ones96[:, :], 1.0)
        zc = cp.tile([C, 1], F32, tag="zc")
        nc.vector.memset(zc[:, :], 0.0)

        def nearest_chunk(srctile, c0, wc, tagp="p1"):
            """Compute clipped nearest (f32) for a chunk of a resident wrapped tile."""
            qch = srctile[:, c0:c0 + wc]
            y = p1.tile([48, CH1], F32, tag=tagp + "a", name="nck_y")
            TS(out=y[:, :wc], in0=qch, scalar1=G_OR, scalar2=G_IV,
               op0=ALU.subtract, op1=ALU.mult)
            n = p1.tile([48, CH1], F32, tag=tagp + "c", name="nck_n")
            TS(out=n[:, :wc], in0=y[:, :wc], scalar1=MAGIC, scalar2=MAGIC,
               op0=ALU.add, op1=ALU.subtract)
            TS(out=n[:, :wc], in0=n[:, :wc], scalar1=0.0, scalar2=float(GRID - 2),
               op0=ALU.max, op1=ALU.min)
            return qch, n

        CH1 = 1728
        qwsh = cp.tile([48, wsh], F32, tag="qwsh")
        nc.sync.dma_start(qwsh[:, :], qwshD[:, :])

        # ===== PHASE 2: gather indices for this core's shard ====
        idx128 = cp.tile([P, wsh], I16, tag="idx128")
        ch = 432
        for h in range(0, wsh, ch):
            wc = min(ch, wsh - h)
            _, nsw = nearest_chunk(qwsh, h, wc, tagp="p2")
            ix_ps = pp.tile([128, M], F32, tag="scr", space="PSUM")
            MMX(ix_ps[0:16, 0:wc], coeff[:, :], nsw[:, :wc], start=True, stop=True)
            nc.vector.tensor_copy(idx128[0:16, h:h + wc], ix_ps[0:16, 0:wc])
        for rep in (16, 32, 64):
            nc.gpsimd.dma_start(out=idx128[rep:2 * rep, :], in_=idx128[0:rep, :])

        # ===== PHASE 1: global rel minima over the FULL query set (wrapped layout) ====
        CH1 = 1728
        nch1 = (wfull + CH1 - 1) // CH1
        qwall = cv("qw")
        qwsh = cp.tile([48, wsh], F32, tag="qwsh")
        nc.sync.dma_start(qwsh[:, :], qwshD[:, :])


        m2ch = p1.tile([48, 2 * nch1], F32, tag="m2ch")
        for chi in range(nch1):
            c0 = chi * CH1
            wc = min(CH1, wfull - c0)
            qch, n = nearest_chunk(qwall, c0, wc)
            cw = p1.tile([48, CH1], F32, tag="p1a")
            TS(out=cw[:, :wc], in0=n[:, :wc], scalar1=G_VX, scalar2=G_OR,
               op0=ALU.mult, op1=ALU.add)
            dw = p1.tile([48, CH1], F32, tag="p1b")
            TT(out=dw[:, :wc], in0=cw[:, :wc], in1=qch, op=ALU.subtract)
            r0 = p1.tile([48, CH1], F32, tag="p1a")
            TS(out=r0[:, :wc], in0=dw[:, :wc], scalar1=G_CL, scalar2=None, op0=ALU.max)
            nc.vector.tensor_reduce(out=m2ch[:, chi:chi + 1], in_=r0[:, :wc],
                                    axis=mybir.AxisListType.X, op=ALU.min)
            r1 = p1.tile([48, CH1], F32, tag="p1c")
            TS(out=r1[:, :wc], in0=dw[:, :wc], scalar1=G_VX, scalar2=G_CL,
               op0=ALU.add, op1=ALU.max)
            nc.vector.tensor_reduce(out=m2ch[:, nch1 + chi:nch1 + chi + 1], in_=r1[:, :wc],
                                    axis=mybir.AxisListType.X, op=ALU.min)
        m2 = p1.tile([48, 2], F32, tag="m2")
        nc.vector.tensor_reduce(out=m2[:, 0:1], in_=m2ch[:, 0:nch1],
                                axis=mybir.AxisListType.X, op=ALU.min)
        nc.vector.tensor_reduce(out=m2[:, 1:2], in_=m2ch[:, nch1:2 * nch1],
                                axis=mybir.AxisListType.X, op=ALU.min)
        mt_ps = pp.tile([128, M], F32, tag="scr", space="PSUM")
        nc.tensor.transpose(out=mt_ps[0:2, 0:48], in_=m2[:, :], identity=ident[0:48, 0:48])
        mts = p1.tile([2, 48], F32, tag="mts")
        ACT(out=mts[:, :], in_=mt_ps[0:2, 0:48], func=AF.Copy)
        mn6 = p1.tile([2, 3], F32, tag="mn6")
        for d in range(3):
            nc.vector.tensor_reduce(out=mn6[:, d:d + 1], in_=mts[:, 16 * d:16 * (d + 1)],
                                    axis=mybir.AxisListType.X, op=ALU.min)
        mnt_ps = pp.tile([128, M], F32, tag="scr", space="PSUM")
        nc.tensor.transpose(out=mnt_ps[0:3, 0:2], in_=mn6[:, :], identity=ident[0:2, 0:2])
        mns = p1.tile([3, 2], F32, tag="mns")
        ACT(out=mns[:, :], in_=mnt_ps[0:3, 0:2], func=AF.Copy)
        # delta_d = (vox + min0 - min1) / (1.5 vox); negm0c = -min0/(1.5 vox)
        dcol = p1.tile([3, 1], F32, tag="dcol")
        TT(out=dcol[:, :], in0=mns[:, 0:1], in1=mns[:, 1:2], op=ALU.subtract)
        TS(out=dcol[:, :], in0=dcol[:, :], scalar1=G_VX, scalar2=G_I15,
           op0=ALU.add, op1=ALU.mult)
        negm0c = p1.tile([3, 1], F32, tag="negm0c")
        TS(out=negm0c[:, :], in0=mns[:, 0:1], scalar1=G_I15, scalar2=-1.0,
           op0=ALU.mult, op1=ALU.mult)
        # offrow [1,128]: delta angle offsets + ones row marker
        or_ps = pp.tile([128, M], F32, tag="scr", space="PSUM")
        MMX(or_ps[0:P, 0:1], emd[:, :], dcol[:, :], start=True, stop=False)
        MMX(or_ps[0:P, 0:1], e105[:, :], ones11[:, :], start=False, stop=True)
        orS = p1.tile([P, 1], F32, tag="orS")
        ACT(out=orS[:, :], in_=or_ps[0:P, 0:1], func=AF.Copy)
        ort_ps = pp.tile([128, M], F32, tag="scr", space="PSUM")
        nc.tensor.transpose(out=ort_ps[0:1, 0:P], in_=orS[:, :], identity=ident[:, :])
        offrow = p1.tile([1, P], F32, tag="offrow")
        ACT(out=offrow[:, :], in_=ort_ps[0:1, 0:P], func=AF.Copy)
        nc.gpsimd.dma_start(out=emstk[96:97, :], in_=offrow[:, :])
        # per-(block,corner) bias rows -> wcoord row NONE via transpose + DMAs
        db_ps = pp.tile([128, M], F32, tag="scr", space="PSUM")
        for cb in range(16):
            MMX(db_ps[0:C, cb:cb + 1], wds[:, cb * C:(cb + 1) * C], dcol[:, :],
                start=True, stop=True)
        biasT = p1.tile([C, 16], F32, tag="biasT")
        TT(out=biasT[:, :], in0=db_ps[0:C, 0:16], in1=hbm[:, :], op=ALU.add)
        bt_ps = pp.tile([128, M], F32, tag="scr", space="PSUM")
        nc.tensor.transpose(out=bt_ps[0:16, 0:C], in_=biasT[:, :], identity=ident[0:C, 0:C])
        biasTT = p1.tile([16, C], BF16, tag="biasTT")
        ACT(out=biasTT[:, :], in_=bt_ps[0:16, 0:C], func=AF.Copy)
        for cb in range(16):
            nc.gpsimd.dma_start(out=wcoord[NONE:NONE + 1, cb * C:(cb + 1) * C],
                                in_=biasTT[cb:cb + 1, :])

        # ===== PHASE 3: main loop ====
        qcv = qcD[:, :]
        for t in range(nt):
            cs = t * M
            GT = wp.tile([P, NCORE, M], BF16, tag="gt")
            nc.gpsimd.dma_gather(
                out_ap=GT[:, :, :], in_ap=tableD[:, :],
                idxs_ap=idx128[:, t * (M // 16):(t + 1) * (M // 16)],
                num_idxs=M, num_idxs_reg=M, elem_size=NCORE * P,
                transpose=True, queue_num=0,
            )
            # stacked ang rhs [97, M] (32-aligned blocks): rows 0:3 rnq,
            # 32:35 qq, 64:67 n3q, 96 ones; gap rows zeroed (lhsT is 0 there,
            # but garbage could be inf/nan).
            stk = wp.tile([97, M], F32, tag="stk")
            nc.vector.memset(stk[:, :], 0.0)
            nc.vector.memset(stk[96:97, :], 1.0)
            nc.sync.dma_start(stk[32:35, :], qcv[:, cs:cs + M])
            qq3 = wp.tile([3, M], F32, tag="qq3")
            nc.sync.dma_start(qq3[:, :], qcv[:, cs:cs + M])
            qq = qq3[:, :]
            yq = wp.tile([3, M], F32, tag="qsA")
            TS(out=yq[:, :], in0=qq, scalar1=G_OR,
               scalar2=G_IV, op0=ALU.subtract, op1=ALU.mult)
            n3q = stk[64:67, :]
            TS(out=n3q, in0=yq[:, :], scalar1=MAGIC, scalar2=MAGIC,
               op0=ALU.add, op1=ALU.subtract)
            TS(out=n3q, in0=n3q, scalar1=0.0, scalar2=float(GRID - 2),
               op0=ALU.max, op1=ALU.min)
            ccq = wp.tile([3, M], F32, tag="qsA")
            TS(out=ccq[:, :], in0=n3q, scalar1=G_VX,
               scalar2=G_OR, op0=ALU.mult, op1=ALU.add)
            r0q = wp.tile([3, M], F32, tag="qsB")
            TT(out=r0q[:, :], in0=ccq[:, :], in1=qq, op=ALU.subtract)
            TS(out=r0q[:, :], in0=r0q[:, :], scalar1=G_CL,
               scalar2=None, op0=ALU.max)
            TS(out=stk[0:3, :], in0=r0q[:, :], scalar1=G_I15,
               scalar2=negm0c[:, 0:1], op0=ALU.mult, op1=ALU.add)
            # stacked trilinear rhs [35, M]: rows 0:3 twq, 32:35 omq
            tom = wp.tile([35, M], F32, tag="twq")
            nc.vector.memset(tom[:, :], 0.0)
            twq = tom[0:3, :]
            TS(out=twq, in0=r0q[:, :], scalar1=-0.5,
               scalar2=-(1.0 - EPS) / 2, op0=ALU.add, op1=ALU.max)
            TS(out=twq, in0=twq, scalar1=(1.0 - EPS) / 2,
               scalar2=0.5, op0=ALU.min, op1=ALU.add)
            TS(out=tom[32:35, :], in0=twq, scalar1=-1.0, scalar2=1.0,
               op0=ALU.mult, op1=ALU.add)

            ang = pp.tile([P, M], F32, tag="ang", space="PSUM")
            MMX(ang[:, :], emstk[:, :], stk[:, :], start=True, stop=True)
            sst = wp.tile([P, M], BF16, tag="sst")
            rr = wp.tile([P, M], F32, tag="rr")
            red = wp.tile([P, M], F32, tag="red")
            TS(out=rr[0:C, :], in0=ang[0:C, :], scalar1=INV2PI, scalar2=MAGIC,
               op0=ALU.mult, op1=ALU.add)
            TS(out=rr[0:C, :], in0=rr[0:C, :], scalar1=MAGIC, scalar2=NEG2PI,
               op0=ALU.subtract, op1=ALU.mult)
            TT(out=red[0:C, :], in0=ang[0:C, :], in1=rr[0:C, :], op=ALU.add)
            ACT(out=sst[0:C, :], in_=red[0:C, :], func=AF.Sin, bias=0.0)
            ACT(out=sst[C:P, :], in_=ang[C:P, :], func=AF.Copy)

            # trilinear factors: F1/F2/F3 [8, M] psums via selection matmuls, then products
            fps = []
            for fi, tg in enumerate(("scr", "wcp", "ang")):
                f_ps = pp.tile([P, M], F32, tag=tg, space="PSUM",
                               name=f"f{fi}_ps")
                MMX(f_ps[0:8, :], sel6[:, fi * 8:(fi + 1) * 8], tom[:, :],
                    start=True, stop=True)
                fps.append(f_ps)
            f2s = wp.tile([8, M], F32, tag="f2s")
            nc.vector.tensor_copy(f2s[:, :], fps[1][0:8, :])
            w12t = wp.tile([8, M], F32, tag="w12t")
            TT(out=w12t[:, :], in0=fps[0][0:8, :], in1=f2s[:, :], op=ALU.mult)
            wct8 = wp.tile([8, M], BF16, tag="wct8")
            TT(out=wct8[:, :], in0=w12t[:, :], in1=fps[2][0:8, :], op=ALU.mult)

            pout = pp.tile([45, M], F32, tag="pout", space="PSUM")
            for cpair in range(4):
                cA = 2 * cpair
                lp0 = pp.tile([C, 2 * M], F32, tag="lp0", space="PSUM")
                for ci in range(2):
                    sl = slice(ci * M, (ci + 1) * M)
                    MMX(lp0[:, sl], wh0[:, :], GT[:, cA + ci, :], start=True, stop=False)
                for ci in range(2):
                    cc_ = cA + ci
                    sl = slice(ci * M, (ci + 1) * M)
                    MMX(lp0[:, sl], wcoord[:, cc_ * C:(cc_ + 1) * C], sst[:, :],
                        start=False, stop=True)
                t1p = wp.tile([C, 2 * M], BF16, tag="t1p", bufs=3)
                ACT(out=t1p[:, :], in_=lp0[:, :], func=act_fn, bias=zc[:, 0:1])
                lp1 = pp.tile([C, 2 * M], F32, tag="lp1", space="PSUM")
                for ci in range(2):
                    sl = slice(ci * M, (ci + 1) * M)
                    MMX(lp1[:, sl], wl["w01"][:, :], t1p[:, sl], start=True, stop=True)
                t2p = wp.tile([C, 2 * M], BF16, tag="t2p", bufs=3)
                ACT(out=t2p[:, :], in_=lp1[:, :], func=act_fn, bias=bl[:, 0:1])
                lp2 = pp.tile([C, 2 * M], F32, tag="lp0", space="PSUM")
                for ci in range(2):
                    sl = slice(ci * M, (ci + 1) * M)
                    MMX(lp2[:, sl], wl["w02"][:, :], t2p[:, sl], start=True, stop=True)
                t3p = wp.tile([C, 2 * M], BF16, tag="t3p", bufs=3)
                ACT(out=t3p[:, :], in_=lp2[:, :], func=act_fn, bias=bl[:, 1:2])
                h1p = wp.tile([C, 2 * M], BF16, tag="h1p")
                TT(out=h1p[:, :], in0=GT[0:C, cA:cA + 2, :], in1=t3p[:, :], op=ALU.add)

                lp3 = pp.tile([C, 2 * M], F32, tag="lp1", space="PSUM")
                for ci in range(2):
                    sl = slice(ci * M, (ci + 1) * M)
                    MMX(lp3[:, sl], wh1[:, :], h1p[:, sl], start=True, stop=False)
                for ci in range(2):
                    cc_ = cA + ci
                    sl = slice(ci * M, (ci + 1) * M)
                    MMX(lp3[:, sl], wcoord[:, (8 + cc_) * C:(9 + cc_) * C], sst[:, :],
                        start=False, stop=True)
                u1p = wp.tile([C, 2 * M], BF16, tag="t1p", bufs=3, name="u1p")
                ACT(out=u1p[:, :], in_=lp3[:, :], func=act_fn, bias=zc[:, 0:1])
                lp4 = pp.tile([C, 2 * M], F32, tag="lp0", space="PSUM")
                for ci in range(2):
                    sl = slice(ci * M, (ci + 1) * M)
                    MMX(lp4[:, sl], wl["w11"][:, :], u1p[:, sl], start=True, stop=True)
                u2p = wp.tile([C, 2 * M], BF16, tag="t2p", bufs=3, name="u2p")
                ACT(out=u2p[:, :], in_=lp4[:, :], func=act_fn, bias=bl[:, 2:3])
                lp5 = pp.tile([C, 2 * M], F32, tag="lp1", space="PSUM")
                for ci in range(2):
                    sl = slice(ci * M, (ci + 1) * M)
                    MMX(lp5[:, sl], wl["w12"][:, :], u2p[:, sl], start=True, stop=True)
                u3p = wp.tile([C, 2 * M], BF16, tag="t3p", bufs=3, name="u3p")
                ACT(out=u3p[:, :], in_=lp5[:, :], func=act_fn, bias=bl[:, 3:4])
                h2p = wp.tile([C, 2 * M], BF16, tag="h2p")
                TT(out=h2p[:, :], in0=h1p[:, :], in1=u3p[:, :], op=ALU.add)

                h2ss = []
                for ci in range(2):
                    cc_ = cA + ci
                    sl = slice(ci * M, (ci + 1) * M)
                    wcp = pp.tile([P, M], F32, tag="wcp", space="PSUM")
                    MMX(wcp[0:C, :], ec8[:, cc_ * C:(cc_ + 1) * C], wct8[:, :],
                        start=True, stop=True)
                    h2s = wp.tile([C, M], BF16, tag=f"h2s{ci}", name=f"h2s{ci}")
                    TT(out=h2s[:, :], in0=h2p[:, sl], in1=wcp[0:C, :], op=ALU.mult)
                    h2ss.append(h2s)
                for ci in range(2):
                    cc_ = cA + ci
                    MMX(pout[:, :], wpost[:, :], h2ss[ci][:, :], start=(cc_ == 0),
                        stop=(cc_ == 7))
            osb = wp.tile([45, M], F32, tag="osb")
            ACT(out=osb[:, :], in_=pout[:, :], func=AF.Identity, bias=postb[:, 0:1])
            nc.sync.dma_start(outD[:, cs:cs + M], osb[:, :])

    from concourse.library_overlay import lower_extended_insts
    lower_extended_insts(nc)
    if legalize:
        _legalize_waits(nc)
    return nc


def _wait_limit(ins):
    return 1


def _legalize_waits(nc):
    """The walrus codegen allows only one sync-wait command per normal
    instruction. Split excess waits onto dedicated same-engine NOPs
    inserted immediately before the instruction (program position is
    unchanged, so dependency semantics are identical)."""
    import bass_rust as _br
    nid = 0
    for fn in nc.m.functions:
        for bb in fn.blocks:
            il = bb.instructions
            i = 0
            while i < len(il):
                ins = il[i]
                si = ins.sync_info
                lim = _wait_limit(ins)
                if si is not None and si.on_wait and len(si.on_wait) > lim:
                    ow = list(si.on_wait)
                    keep, excess = ow[-lim:], ow[:-lim]
                    for w in excess:
                        nid += 1
                        nop = mybir.InstNoOp(name=f"legwait-{nid}", ins=[], outs=[])
                        nop.engine = ins.engine
                        nop.sync_info = _br.SyncInfo(on_wait=[w], on_update=[])
                        il.insert(i, nop)
                        i += 1
                    si.on_wait = keep
                i += 1


def make_in_maps(inputs, cst=None, nsh=NSH, ncore=NCORE):
    shared, percore = make_split_maps(inputs, cst, nsh, ncore)
    return [dict(shared, **pc) for pc in percore]


def make_split_maps(inputs, cst=None, nsh=NSH, ncore=NCORE):
    cst = cst or host_prep(inputs)
    qflat = cst["qflat"]
    wsh = nsh // 16
    wfull = cst["qw"].shape[1]
    lay, ncols = blob_layout(wfull)
    cblob = np.zeros((P, ncols), np.float32)
    for nm, (o, r, cc) in lay.items():
        arr = cst[nm]
        assert arr.shape == (r, cc), (nm, arr.shape, (r, cc))
        cblob[0:r, o:o + cc] = arr
    nb = sum(cc for _, _, cc in BLAY_BF)
    bblob = np.zeros((P, nb), BF)
    _o = 0
    for nm, r, cc in BLAY_BF:
        bblob[0:r, _o:_o + cc] = cst[nm]
        _o += cc
    shared = dict(table=np.ascontiguousarray(cst["table"]),
                  bblob=np.ascontiguousarray(bblob),
                  cblob=np.ascontiguousarray(cblob))
    percore = []
    for core in range(ncore):
        sl = qflat[:, core * nsh:(core + 1) * nsh]
        percore.append(dict(
            qc=np.ascontiguousarray(sl),
            qwsh=np.ascontiguousarray(
                sl.reshape(3, wsh, 16).transpose(0, 2, 1).reshape(48, wsh)),
        ))
    return shared, percore


_CACHE = {}


def get_runner(nc, ncore=NCORE):
    """Compile an SPMD runner: shared inputs replicated (one transfer),
    per-core inputs sharded along axis 0."""
    import concourse.bass2jax as b2j
    import jax
    from jax.sharding import Mesh, PartitionSpec, NamedSharding
    from jax.experimental.shard_map import shard_map

    b2j.install_neuronx_cc_hook()
    partition_name = nc.partition_id_tensor.name if nc.partition_id_tensor else None
    in_names, out_names, out_avals, zero_outs = [], [], [], []
    for alloc in nc.m.functions[0].allocations:
        if not isinstance(alloc, mybir.MemoryLocationSet):
            continue
        name = alloc.memorylocations[0].name
        if alloc.kind == "ExternalInput":
            if name != partition_name:
                in_names.append(name)
        elif alloc.kind == "ExternalOutput":
            out_names.append(name)
            shape = tuple(alloc.tensor_shape)
            dtype = mybir.dt.np(alloc.dtype)
            out_avals.append(jax.core.ShapedArray(shape, dtype))
            zero_outs.append(np.zeros(shape, dtype))
    n_params = len(in_names)
    all_names = in_names + out_names
    if partition_name is not None:
        all_names.append(partition_name)

    def _body(*args):
        operands = list(args)
        if partition_name is not None:
            operands.append(b2j.partition_id_tensor())
        outs = b2j._bass_exec_p.bind(
            *operands, out_avals=tuple(out_avals), in_names=tuple(all_names),
            out_names=tuple(out_names), lowering_input_output_aliases=(),
            sim_require_finite=True, sim_require_nnan=True, nc=nc)
        return tuple(outs)

    devices = jax.devices()[:ncore]
    mesh = Mesh(np.asarray(devices), ("core",))

    def runner(shared, percore):
        specs, arrs = [], []
        for nm in in_names:
            if nm in shared:
                specs.append(PartitionSpec())
                arrs.append(shared[nm])
            else:
                specs.append(PartitionSpec("core"))
                arrs.append(np.concatenate([pc[nm] for pc in percore], axis=0))
        for z in zero_outs:
            specs.append(PartitionSpec("core"))
            arrs.append(np.concatenate([z] * ncore, axis=0))
        fn = jax.jit(shard_map(_body, mesh=mesh, in_specs=tuple(specs),
                               out_specs=(PartitionSpec("core"),) * len(out_names),
                               check_rep=False), keep_unused=True)
        dev = [jax.device_put(a, NamedSharding(mesh, s)) for a, s in zip(arrs, specs)]
        outs = fn(*dev)
        jax.block_until_ready(outs)
        return {nm: np.asarray(o) for nm, o in zip(out_names, outs)}, (fn, dev)

    return runner


def kernel(**inputs):
    cst = host_prep(inputs)
    if "nc" not in _CACHE:
        _CACHE["nc"] = build(geo=cst["geo"])
        _CACHE["runner"] = get_runner(_CACHE["nc"])
    shared, percore = make_split_maps(inputs, cst)
    outmap, _ = _CACHE["runner"](shared, percore)
    full_cat = outmap["out"]                     # (8*45, NSH)
    full = np.concatenate([full_cat[c * 45:(c + 1) * 45] for c in range(NCORE)], axis=1)
    return full.reshape(1, 45, QE, QE, QE).astype(np.float32)



# revision 20
# speedup vs baseline: 1.2299x; 1.0394x over previous
"""Trainium2 Bass kernel for nn_ContRepDecoder (8-core SPMD, data-parallel over query points).

Strategy:
- Host builds a (32768, 1024) bf16 table: row v = concat over the 8 cube corners of the
  96 context features (zero-padded to 128) at voxel v+offset.
- Each core gathers its points' rows with dma_gather(transpose=True) -> feature-major
  [128, 8, M] tiles (corner-blocked), no on-chip transposes.
- Coordinate features (q, cc, rel_norm, fourier enc) are assembled into one f32
  "coordinate stack" SST [128, M] via tiny-K matmuls into PSUM + ACT Sin + range reduce.
- MLP: per corner 2 matmuls per L0 (gathered h + per-corner masked coord lhsT),
  silu on 2-corner psum pairs, residuals on DVE, trilinear combine via outer-product
  weight psum + pre-scaled post matmuls accumulating into one [45, M] psum.
- Offset-1 coordinate features are affine in offset-0 ones; the differences are folded
  into per-corner bias rows (host weights) + device-computed delta terms from the
  global rel minima (computed once per core over the FULL query set).
"""
import sys

for _p in ("/opt/trn_rl_repo",):
    if _p not in sys.path:
        sys.path.insert(0, _p)

import numpy as np
import ml_dtypes

import concourse.bass as bass
import concourse.mybir as mybir
import concourse.tile as tile

P = 128
C = 96
NFRQ = 8
GRID = 32
QE = 48
NPTS = QE ** 3
NCORE = 8
NSH = NPTS // NCORE
M = 512
EPS = 1e-7
PI = float(np.pi)
F32 = mybir.dt.float32
BF16 = mybir.dt.bfloat16
I16 = mybir.dt.int16
AF = mybir.ActivationFunctionType
ALU = mybir.AluOpType

# SST partition layout
NSIN, NCOS, NQ, NCC, NRN, NONE = 0, 48, 96, 99, 102, 105
BF = ml_dtypes.bfloat16


def corner_off(c):
    return (c >> 2 & 1, c >> 1 & 1, c & 1)


def blob_layout(wfull):
    """(name, rows, cols) entries packed along the free dim of one [128, W] f32 blob."""
    ents = [("hbm", C, 16), ("wdsm", 3, 16 * C),
            ("bl", C, 4), ("postb", 45, 1), ("emstk", 97, P),
            ("emd", 3, P), ("e105", 1, P),
            ("coeffidx", 48, 16),
            ("ident", P, P), ("qw", 48, wfull)]
    off, lay = 0, {}
    for nm, r, cc in ents:
        lay[nm] = (off, r, cc)
        off += cc
    return lay, off


BLAY_BF = [("wh0", P, C), ("wh1", C, C), ("w01", C, C), ("w02", C, C),
           ("w11", C, C), ("w12", C, C), ("wpost", C, 45),
           ("wcoord", P, 16 * C), ("sel6", 35, 24), ("ec8", 8, 8 * C)]
QOFF = 31.0


def _enc_row(d, part, k):
    # row of enc feature (dim d, part 0=sin 1=cos, freq k) within the 60-row coord block
    return 12 + d * 16 + part * 8 + k


def host_prep(inputs):
    cse = np.asarray(inputs["context_spatial_extent"], np.float32)
    origin = cse[0, :, 0, 0, 0].copy()
    vox = np.abs(cse[0, :, 1, 1, 1] - cse[0, :, 0, 0, 0])
    qvs = np.asarray(inputs["query_vox_size"], np.float32)[0]
    qflat = np.asarray(inputs["query_coord"], np.float32)[0].reshape(3, NPTS)

    ctx_t = np.moveaxis(np.asarray(inputs["context_v"], np.float32)[0], 0, -1)
    tmp = np.zeros((33, 33, 33, C), np.float32)
    tmp[:32, :32, :32] = ctx_t
    table = np.zeros((GRID ** 3, NCORE * P), np.float32)
    for c in range(8):
        i, j, k = corner_off(c)
        table[:, c * P:c * P + C] = tmp[i:i + 32, j:j + 32, k:k + 32, :].reshape(GRID ** 3, C)
    table = table.astype(BF)

    wfull = NPTS // 16
    qw = qflat.reshape(3, wfull, 16).transpose(0, 2, 1).reshape(48, wfull).copy()

    freqs = (5.0 ** (np.arange(NFRQ) / NFRQ)).astype(np.float32)
    ws = {k: np.asarray(v, np.float32) for k, v in inputs.items() if k.startswith(("blk", "post"))}

    def coord_lhsT(w0, b0):
        # sst coord-row basis (bf16-safe): v1 = vox*n - QOFF (exact odd ints),
        # v2 = cc0 - q  (in [-1,1]).  q = v1 - v2 + QOFF ; cc0 = v1 + QOFF.
        wc = w0[C:, :]
        out = np.zeros((8, P, C), np.float32)
        hostbias = np.zeros((8, C), np.float32)
        wds = np.zeros((8, 3, C), np.float32)
        for c in range(8):
            off = corner_off(c)
            hb = b0 + qvs @ wc[0:3]
            for d in range(3):
                o = off[d]
                for k in range(NFRQ):
                    out[c, NSIN + (2 * d + o) * 8 + k, :] = wc[_enc_row(d, 0, k)]
                    out[c, NCOS + (2 * d + o) * 8 + k, :] = wc[_enc_row(d, 1, k)]
                out[c, NQ + d, :] = wc[6 + d] + wc[3 + d]
                out[c, NCC + d, :] = -wc[6 + d]
                out[c, NRN + d, :] = wc[9 + d]
                wds[c, d, :] = o * wc[9 + d]
                hb = hb + (off[d] * float(vox[d])) * wc[3 + d] \
                        + QOFF * (wc[6 + d] + wc[3 + d])
            hostbias[c] = hb
        return out, hostbias, wds

    c0, hb0, wd0 = coord_lhsT(ws["blk0_w0"], ws["blk0_b0"])
    c1, hb1, wd1 = coord_lhsT(ws["blk1_w0"], ws["blk1_b0"])
    wcoord = np.concatenate([c0, c1], 0).transpose(1, 0, 2).reshape(P, 16 * C).copy()
    hbm = np.concatenate([hb0, hb1], 0).T.copy()
    wdsm = np.concatenate([wd0, wd1], 0).transpose(1, 0, 2).reshape(3, 16 * C).copy()

    wh0 = np.zeros((P, C), np.float32)
    wh0[:C] = ws["blk0_w0"][:C]

    # ang matmul lhsT stack [97, P] (32-aligned blocks for DVE partition-start
    # legality): rows 0-2 emr (rhs=rnq), 32-34 emq (rhs=qq), 64-66 emn
    # (rhs=n3q), row 96 = runtime offrow (rhs=ones).
    emr = np.zeros((3, P), np.float32)
    emq = np.zeros((3, P), np.float32)
    emn = np.zeros((3, P), np.float32)
    emd = np.zeros((3, P), np.float32)
    for d in range(3):
        for o in (0, 1):
            for k in range(NFRQ):
                w = 2 * PI * freqs[k]
                emr[d, NSIN + (2 * d + o) * 8 + k] = w
                emr[d, NCOS + (2 * d + o) * 8 + k] = w
                if o == 1:
                    emd[d, NSIN + (2 * d + 1) * 8 + k] = w
                    emd[d, NCOS + (2 * d + 1) * 8 + k] = w
        emr[d, NRN + d] = 1.0
        emq[d, NCC + d] = -1.0                 # v2 = vox*n - q
        emn[d, NCC + d] = float(vox[d])
        emn[d, NQ + d] = float(vox[d])         # v1 = vox*n - QOFF
    e105 = np.zeros((1, P), np.float32)
    e105[0, NONE] = 1.0
    e105[0, NCOS:NCOS + 48] = 0.5 * PI   # cos = sin(x + pi/2), folded into psum consts
    e105[0, NQ:NQ + 3] = -QOFF
    emstk = np.zeros((97, P), np.float32)
    emstk[0:3] = emr
    emstk[32:35] = emq
    emstk[64:67] = emn

    coeffidx = np.zeros((48, 16), np.float32)
    for ch, w in enumerate((GRID * GRID, GRID, 1)):
        for j in range(16):
            coeffidx[ch * 16 + j, j] = w
    ident = np.eye(P, dtype=np.float32)

    # trilinear factor selection [35, 24]: 3 blocks of [35,8]; rows 0:3 pick
    # from twq, rows 32:35 from omq (32-aligned stacked rhs).
    # F1 (x-offset i) uses t-row 2 (tx), F2 (j) row 1, F3 (k) row 0
    sel6 = np.zeros((35, 24), np.float32)
    for c in range(8):
        i, j, k = corner_off(c)
        sel6[2, 0 * 8 + c] = i       # F1 from tw
        sel6[34, 0 * 8 + c] = 1 - i  # F1 from om
        sel6[1, 1 * 8 + c] = j
        sel6[33, 1 * 8 + c] = 1 - j
        sel6[0, 2 * 8 + c] = k
        sel6[32, 2 * 8 + c] = 1 - k

    ec8 = np.zeros((8, 8 * C), np.float32)
    for c in range(8):
        ec8[c, c * C:(c + 1) * C] = 1.0

    # isotropic geometry -> immediate scalars
    assert np.allclose(vox, vox[0]) and np.allclose(origin, origin[0])
    geo = dict(orig=float(origin[0]), ivox=float(np.float32(1.0) / vox[0]),
               vox=float(vox[0]), clamp=float(-vox[0] / 2 + np.float32(EPS)),
               i15=float(np.float32(1.0) / (np.float32(1.5) * vox[0])))

    return dict(
        table=table, qw=qw, qflat=qflat,
        wcoord=wcoord.astype(BF), hbm=hbm, wdsm=wdsm,
        wh0=wh0.astype(BF), wh1=ws["blk1_w0"][:C].astype(BF),
        w01=ws["blk0_w1"].astype(BF), w02=ws["blk0_w2"].astype(BF),
        w11=ws["blk1_w1"].astype(BF), w12=ws["blk1_w2"].astype(BF),
        wpost=ws["post_w"].astype(BF),
        bl=np.stack([ws["blk0_b1"], ws["blk0_b2"], ws["blk1_b1"], ws["blk1_b2"]], 1).copy(),
        postb=ws["post_b"][:, None].copy(),
        emstk=emstk, emd=emd, e105=e105,
        coeffidx=coeffidx, ident=ident, sel6=sel6.astype(BF), ec8=ec8.astype(BF), geo=geo,
        origin=origin, vox=vox,
    )


def build(nsh=NSH, nfull=NPTS, act_silu=True, geo=None, legalize=True):
    geo = geo or dict(orig=0.0, ivox=0.5, vox=2.0, clamp=float(-1.0 + np.float32(EPS)), i15=float(np.float32(1.0) / np.float32(3.0)))
    G_OR, G_IV, G_VX, G_CL, G_I15 = geo["orig"], geo["ivox"], geo["vox"], geo["clamp"], geo["i15"]
    MAGIC = float(2.0 ** 23)
    INV2PI = float(np.float32(1.0) / np.float32(2 * PI))
    NEG2PI = float(-2 * PI)
    wfull = nfull // 16
    wsh = nsh // 16
    nt = nsh // M
    act_fn = AF.Silu if act_silu else AF.Tanh

    lay, ncols = blob_layout(wfull)
    nbcols = sum(cc for _, _, cc in BLAY_BF)
    nc = bass.Bass()
    dp = nc.declare_dram_parameter
    tableD = dp("table", [GRID ** 3, NCORE * P], BF16, isOutput=False)
    qcD = dp("qc", [3, nsh], F32, isOutput=False)
    qwshD = dp("qwsh", [48, wsh], F32, isOutput=False)
    cblobD = dp("cblob", [P, ncols], F32, isOutput=False)
    bblobD = dp("bblob", [P, nbcols], BF16, isOutput=False)
    outD = dp("out", [45, nsh], F32, isOutput=True)

    TS, TT = nc.vector.tensor_scalar, nc.vector.tensor_tensor
    MMX = nc.tensor.matmul
    ACT = nc.scalar.activation

    from contextlib import ExitStack
    with tile.TileContext(nc) as tc, ExitStack() as es:
        cp = es.enter_context(tc.tile_pool(name="const", bufs=1))
        p1 = es.enter_context(tc.tile_pool(name="ph1", bufs=1))
        wp = es.enter_context(tc.tile_pool(name="work", bufs=2))
        pp = es.enter_context(tc.tile_pool(name="ps", bufs=1, space="PSUM"))

        from concourse import library_config
        nc.gpsimd.load_library(library_config.mlp)

        cbt = cp.tile([P, ncols], F32, tag="cblob")
        nc.sync.dma_start(cbt[:, :], cblobD[:, :])
        bb = cp.tile([P, nbcols], BF16, tag="bblob")
        nc.sync.dma_start(bb[:, :], bblobD[:, :])

        def cv(nm):
            o, r, cc = lay[nm]
            return cbt[0:r, o:o + cc]
        bf_off = {}
        _o = 0
        for nm, r, cc in BLAY_BF:
            bf_off[nm] = (_o, r, cc)
            _o += cc

        def bv(nm):
            o, r, cc = bf_off[nm]
            return bb[0:r, o:o + cc]
        wcoord = bv("wcoord")
        hbm = cv("hbm")
        wds = cv("wdsm")
        bl = cv("bl")
        postb = cv("postb")
        emstk = cv("emstk")
        emd = cv("emd")
        e105 = cv("e105")
        coeff = cv("coeffidx")
        sel6 = bv("sel6")
        ec8 = bv("ec8")
        ident = cv("ident")
        wh0 = bv("wh0")
        wh1 = bv("wh1")
        wl = {k: bv(k) for k in ("w01", "w02", "w11", "w12")}
        wpost = bv("wpost")
        ones1 = cp.tile([1, M], F32, tag="ones1")
        nc.vector.memset(ones1[:, :], 1.0)
        ones11 = cp.tile([1, 1], F32, tag="ones11")
        nc.vector.memset(ones11[:, :], 1.0)
        ones96 = cp.tile([1, C], F32, tag="ones96")
        nc.vector.memset(ones96[:, :], 1.0)
        zc = cp.tile([C, 1], F32, tag="zc")
        nc.vector.memset(zc[:, :], 0.0)

        def nearest_chunk(srctile, c0, wc, tagp="p1"):
            """Compute clipped nearest (f32) for a chunk of a resident wrapped tile."""
            qch = srctile[:, c0:c0 + wc]
            y = p1.tile([48, CH1], F32, tag=tagp + "a", name="nck_y")
            TS(out=y[:, :wc], in0=qch, scalar1=G_OR, scalar2=G_IV,
               op0=ALU.subtract, op1=ALU.mult)
            n = p1.tile([48, CH1], F32, tag=tagp + "c", name="nck_n")
            TS(out=n[:, :wc], in0=y[:, :wc], scalar1=MAGIC, scalar2=MAGIC,
               op0=ALU.add, op1=ALU.subtract)
            TS(out=n[:, :wc], in0=n[:, :wc], scalar1=0.0, scalar2=float(GRID - 2),
               op0=ALU.max, op1=ALU.min)
            return qch, n

        CH1 = 1728
        qwsh = cp.tile([48, wsh], F32, tag="qwsh")
        nc.sync.dma_start(qwsh[:, :], qwshD[:, :])

        # ===== PHASE 2: gather indices for this core's shard ====
        idx128 = cp.tile([P, wsh], I16, tag="idx128")
        ch = 432
        for h in range(0, wsh, ch):
            wc = min(ch, wsh - h)
            _, nsw = nearest_chunk(qwsh, h, wc, tagp="p2")
            ix_ps = pp.tile([128, M], F32, tag="scr", space="PSUM")
            MMX(ix_ps[0:16, 0:wc], coeff[:, :], nsw[:, :wc], start=True, stop=True)
            nc.vector.tensor_copy(idx128[0:16, h:h + wc], ix_ps[0:16, 0:wc])
        for rep in (16, 32, 64):
            nc.gpsimd.dma_start(out=idx128[rep:2 * rep, :], in_=idx128[0:rep, :])

        # ===== PHASE 1: global rel minima over the FULL query set (wrapped layout) ====
        CH1 = 1728
        nch1 = (wfull + CH1 - 1) // CH1
        qwall = cv("qw")
        qwsh = cp.tile([48, wsh], F32, tag="qwsh")
        nc.sync.dma_start(qwsh[:, :], qwshD[:, :])


        m2ch = p1.tile([48, 2 * nch1], F32, tag="m2ch")
        for chi in range(nch1):
            c0 = chi * CH1
            wc = min(CH1, wfull - c0)
            qch, n = nearest_chunk(qwall, c0, wc)
            cw = p1.tile([48, CH1], F32, tag="p1a")
            TS(out=cw[:, :wc], in0=n[:, :wc], scalar1=G_VX, scalar2=G_OR,
               op0=ALU.mult, op1=ALU.add)
            dw = p1.tile([48, CH1], F32, tag="p1b")
            TT(out=dw[:, :wc], in0=cw[:, :wc], in1=qch, op=ALU.subtract)
            r0 = p1.tile([48, CH1], F32, tag="p1a")
            TS(out=r0[:, :wc], in0=dw[:, :wc], scalar1=G_CL, scalar2=None, op0=ALU.max)
            nc.vector.tensor_reduce(out=m2ch[:, chi:chi + 1], in_=r0[:, :wc],
                                    axis=mybir.AxisListType.X, op=ALU.min)
            r1 = p1.tile([48, CH1], F32, tag="p1c")
            TS(out=r1[:, :wc], in0=dw[:, :wc], scalar1=G_VX, scalar2=G_CL,
               op0=ALU.add, op1=ALU.max)
            nc.vector.tensor_reduce(out=m2ch[:, nch1 + chi:nch1 + chi + 1], in_=r1[:, :wc],
                                    axis=mybir.AxisListType.X, op=ALU.min)
        m2 = p1.tile([48, 2], F32, tag="m2")
        nc.vector.tensor_reduce(out=m2[:, 0:1], in_=m2ch[:, 0:nch1],
                                axis=mybir.AxisListType.X, op=ALU.min)
        nc.vector.tensor_reduce(out=m2[:, 1:2], in_=m2ch[:, nch1:2 * nch1],
                                axis=mybir.AxisListType.X, op=ALU.min)
        mt_ps = pp.tile([128, M], F32, tag="scr", space="PSUM")
        nc.tensor.transpose(out=mt_ps[0:2, 0:48], in_=m2[:, :], identity=ident[0:48, 0:48])
        mts = p1.tile([2, 48], F32, tag="mts")
        ACT(out=mts[:, :], in_=mt_ps[0:2, 0:48], func=AF.Copy)
        mn6 = p1.tile([2, 3], F32, tag="mn6")
        for d in range(3):
            nc.vector.tensor_reduce(out=mn6[:, d:d + 1], in_=mts[:, 16 * d:16 * (d + 1)],
                                    axis=mybir.AxisListType.X, op=ALU.min)
        mnt_ps = pp.tile([128, M], F32, tag="scr", space="PSUM")
        nc.tensor.transpose(out=mnt_ps[0:3, 0:2], in_=mn6[:, :], identity=ident[0:2, 0:2])
        mns = p1.tile([3, 2], F32, tag="mns")
        ACT(out=mns[:, :], in_=mnt_ps[0:3, 0:2], func=AF.Copy)
        # delta_d = (vox + min0 - min1) / (1.5 vox); negm0c = -min0/(1.5 vox)
        dcol = p1.tile([3, 1], F32, tag="dcol")
        TT(out=dcol[:, :], in0=mns[:, 0:1], in1=mns[:, 1:2], op=ALU.subtract)
        TS(out=dcol[:, :], in0=dcol[:, :], scalar1=G_VX, scalar2=G_I15,
           op0=ALU.add, op1=ALU.mult)
        negm0c = p1.tile([3, 1], F32, tag="negm0c")
        TS(out=negm0c[:, :], in0=mns[:, 0:1], scalar1=G_I15, scalar2=-1.0,
           op0=ALU.mult, op1=ALU.mult)
        # offrow [1,128]: delta angle offsets + ones row marker
        or_ps = pp.tile([128, M], F32, tag="scr", space="PSUM")
        MMX(or_ps[0:P, 0:1], emd[:, :], dcol[:, :], start=True, stop=False)
        MMX(or_ps[0:P, 0:1], e105[:, :], ones11[:, :], start=False, stop=True)
        orS = p1.tile([P, 1], F32, tag="orS")
        ACT(out=orS[:, :], in_=or_ps[0:P, 0:1], func=AF.Copy)
        ort_ps = pp.tile([128, M], F32, tag="scr", space="PSUM")
        nc.tensor.transpose(out=ort_ps[0:1, 0:P], in_=orS[:, :], identity=ident[:, :])
        offrow = p1.tile([1, P], F32, tag="offrow")
        ACT(out=offrow[:, :], in_=ort_ps[0:1, 0:P], func=AF.Copy)
        nc.gpsimd.dma_start(out=emstk[96:97, :], in_=offrow[:, :])
        # per-(block,corner) bias rows -> wcoord row NONE via transpose + DMAs
        db_ps = pp.tile([128, M], F32, tag="scr", space="PSUM")
        for cb in range(16):
            MMX(db_ps[0:C, cb:cb + 1], wds[:, cb * C:(cb + 1) * C], dcol[:, :],
                start=True, stop=True)
        biasT = p1.tile([C, 16], F32, tag="biasT")
        TT(out=biasT[:, :], in0=db_ps[0:C, 0:16], in1=hbm[:, :], op=ALU.add)
        bt_ps = pp.tile([128, M], F32, tag="scr", space="PSUM")
        nc.tensor.transpose(out=bt_ps[0:16, 0:C], in_=biasT[:, :], identity=ident[0:C, 0:C])
        biasTT = p1.tile([16, C], BF16, tag="biasTT")
        ACT(out=biasTT[:, :], in_=bt_ps[0:16, 0:C], func=AF.Copy)
        for cb in range(16):
            nc.gpsimd.dma_start(out=wcoord[NONE:NONE + 1, cb * C:(cb + 1) * C],
                                in_=biasTT[cb:cb + 1, :])

        # ===== PHASE 3: main loop ====
        qcv = qcD[:, :]
        for t in range(nt):
            cs = t * M
            GT = wp.tile([P, NCORE, M], BF16, tag="gt")
            nc.gpsimd.dma_gather(
                out_ap=GT[:, :, :], in_ap=tableD[:, :],
                idxs_ap=idx128[:, t * (M // 16):(t + 1) * (M // 16)],
                num_idxs=M, num_idxs_reg=M, elem_size=NCORE * P,
                transpose=True, queue_num=0,
            )
            # stacked ang rhs [97, M] (32-aligned blocks): rows 0:3 rnq,
            # 32:35 qq, 64:67 n3q, 96 ones; gap rows zeroed (lhsT is 0 there,
            # but garbage could be inf/nan).
            stk = wp.tile([97, M], F32, tag="stk")
            if t < 2:
                nc.vector.memset(stk[:, :], 0.0)
                nc.vector.memset(stk[96:97, :], 1.0)
            nc.sync.dma_start(stk[32:35, :], qcv[:, cs:cs + M])
            qq3 = wp.tile([3, M], F32, tag="qq3")
            nc.sync.dma_start(qq3[:, :], qcv[:, cs:cs + M])
            qq = qq3[:, :]
            yq = wp.tile([3, M], F32, tag="qsA")
            TS(out=yq[:, :], in0=qq, scalar1=G_OR,
               scalar2=G_IV, op0=ALU.subtract, op1=ALU.mult)
            n3q = stk[64:67, :]
            TS(out=n3q, in0=yq[:, :], scalar1=MAGIC, scalar2=MAGIC,
               op0=ALU.add, op1=ALU.subtract)
            TS(out=n3q, in0=n3q, scalar1=0.0, scalar2=float(GRID - 2),
               op0=ALU.max, op1=ALU.min)
            ccq = wp.tile([3, M], F32, tag="qsA")
            TS(out=ccq[:, :], in0=n3q, scalar1=G_VX,
               scalar2=G_OR, op0=ALU.mult, op1=ALU.add)
            r0q = wp.tile([3, M], F32, tag="qsB")
            TT(out=r0q[:, :], in0=ccq[:, :], in1=qq, op=ALU.subtract)
            TS(out=r0q[:, :], in0=r0q[:, :], scalar1=G_CL,
               scalar2=None, op0=ALU.max)
            TS(out=stk[0:3, :], in0=r0q[:, :], scalar1=G_I15,
               scalar2=negm0c[:, 0:1], op0=ALU.mult, op1=ALU.add)
            # stacked trilinear rhs [35, M]: rows 0:3 twq, 32:35 omq
            tom = wp.tile([35, M], BF16, tag="twq")
            if t < 2:
                nc.vector.memset(tom[:, :], 0.0)
            twq = tom[0:3, :]
            TS(out=twq, in0=r0q[:, :], scalar1=-0.5,
               scalar2=-(1.0 - EPS) / 2, op0=ALU.add, op1=ALU.max)
            TS(out=twq, in0=twq, scalar1=(1.0 - EPS) / 2,
               scalar2=0.5, op0=ALU.min, op1=ALU.add)
            TS(out=tom[32:35, :], in0=twq, scalar1=-1.0, scalar2=1.0,
               op0=ALU.mult, op1=ALU.add)

            ang = pp.tile([P, M], F32, tag="ang", space="PSUM")
            MMX(ang[:, :], emstk[:, :], stk[:, :], start=True, stop=True)
            sst = wp.tile([P, M], BF16, tag="sst")
            rr = wp.tile([P, M], F32, tag="rr")
            red = wp.tile([P, M], F32, tag="red")
            TS(out=rr[0:C, :], in0=ang[0:C, :], scalar1=INV2PI, scalar2=MAGIC,
               op0=ALU.mult, op1=ALU.add)
            TS(out=rr[0:C, :], in0=rr[0:C, :], scalar1=MAGIC, scalar2=NEG2PI,
               op0=ALU.subtract, op1=ALU.mult)
            TT(out=red[0:C, :], in0=ang[0:C, :], in1=rr[0:C, :], op=ALU.add)
            ACT(out=sst[0:C, :], in_=red[0:C, :], func=AF.Sin, bias=0.0)
            ACT(out=sst[C:P, :], in_=ang[C:P, :], func=AF.Copy)

            # trilinear factors: F1/F2/F3 [8, M] psums via selection matmuls, then products
            fps = []
            for fi, tg in enumerate(("scr", "wcp", "ang")):
                f_ps = pp.tile([P, M], F32, tag=tg, space="PSUM",
                               name=f"f{fi}_ps")
                MMX(f_ps[0:8, :], sel6[:, fi * 8:(fi + 1) * 8], tom[:, :],
                    start=True, stop=True)
                fps.append(f_ps)
            f2s = wp.tile([8, M], F32, tag="f2s")
            nc.vector.tensor_copy(f2s[:, :], fps[1][0:8, :])
            w12t = wp.tile([8, M], F32, tag="w12t")
            TT(out=w12t[:, :], in0=fps[0][0:8, :], in1=f2s[:, :], op=ALU.mult)
            wct8 = wp.tile([8, M], BF16, tag="wct8")
            TT(out=wct8[:, :], in0=w12t[:, :], in1=fps[2][0:8, :], op=ALU.mult)

            pout = pp.tile([45, M], F32, tag="pout", space="PSUM")
            for cpair in range(4):
                cA = 2 * cpair
                lp0 = pp.tile([C, 2 * M], F32, tag="lp0", space="PSUM")
                for ci in range(2):
                    sl = slice(ci * M, (ci + 1) * M)
                    MMX(lp0[:, sl], wh0[:, :], GT[:, cA + ci, :], start=True, stop=False)
                for ci in range(2):
                    cc_ = cA + ci
                    sl = slice(ci * M, (ci + 1) * M)
                    MMX(lp0[:, sl], wcoord[:, cc_ * C:(cc_ + 1) * C], sst[:, :],
                        start=False, stop=True)
                t1p = wp.tile([C, 2 * M], BF16, tag="t1p", bufs=3)
                ACT(out=t1p[:, :], in_=lp0[:, :], func=act_fn, bias=zc[:, 0:1])
                lp1 = pp.tile([C, 2 * M], F32, tag="lp1", space="PSUM")
                for ci in range(2):
                    sl = slice(ci * M, (ci + 1) * M)
                    MMX(lp1[:, sl], wl["w01"][:, :], t1p[:, sl], start=True, stop=True)
                t2p = wp.tile([C, 2 * M], BF16, tag="t2p", bufs=3)
                ACT(out=t2p[:, :], in_=lp1[:, :], func=act_fn, bias=bl[:, 0:1])
                lp2 = pp.tile([C, 2 * M], F32, tag="lp0", space="PSUM")
                for ci in range(2):
                    sl = slice(ci * M, (ci + 1) * M)
                    MMX(lp2[:, sl], wl["w02"][:, :], t2p[:, sl], start=True, stop=True)
                t3p = wp.tile([C, 2 * M], BF16, tag="t3p", bufs=3)
                ACT(out=t3p[:, :], in_=lp2[:, :], func=act_fn, bias=bl[:, 1:2])
                h1p = wp.tile([C, 2 * M], BF16, tag="h1p")
                TT(out=h1p[:, :], in0=GT[0:C, cA:cA + 2, :], in1=t3p[:, :], op=ALU.add)

                lp3 = pp.tile([C, 2 * M], F32, tag="lp1", space="PSUM")
                for ci in range(2):
                    sl = slice(ci * M, (ci + 1) * M)
                    MMX(lp3[:, sl], wh1[:, :], h1p[:, sl], start=True, stop=False)
                for ci in range(2):
                    cc_ = cA + ci
                    sl = slice(ci * M, (ci + 1) * M)
                    MMX(lp3[:, sl], wcoord[:, (8 + cc_) * C:(9 + cc_) * C], sst[:, :],
                        start=False, stop=True)
                u1p = wp.tile([C, 2 * M], BF16, tag="t1p", bufs=3, name="u1p")
                ACT(out=u1p[:, :], in_=lp3[:, :], func=act_fn, bias=zc[:, 0:1])
                lp4 = pp.tile([C, 2 * M], F32, tag="lp0", space="PSUM")
                for ci in range(2):
                    sl = slice(ci * M, (ci + 1) * M)
                    MMX(lp4[:, sl], wl["w11"][:, :], u1p[:, sl], start=True, stop=True)
                u2p = wp.tile([C, 2 * M], BF16, tag="t2p", bufs=3, name="u2p")
                ACT(out=u2p[:, :], in_=lp4[:, :], func=act_fn, bias=bl[:, 2:3])
                lp5 = pp.tile([C, 2 * M], F32, tag="lp1", space="PSUM")
                for ci in range(2):
                    sl = slice(ci * M, (ci + 1) * M)
                    MMX(lp5[:, sl], wl["w12"][:, :], u2p[:, sl], start=True, stop=True)
                u3p = wp.tile([C, 2 * M], BF16, tag="t3p", bufs=3, name="u3p")
                ACT(out=u3p[:, :], in_=lp5[:, :], func=act_fn, bias=bl[:, 3:4])
                h2p = wp.tile([C, 2 * M], BF16, tag="h2p")
                TT(out=h2p[:, :], in0=h1p[:, :], in1=u3p[:, :], op=ALU.add)

                h2ss = []
                for ci in range(2):
                    cc_ = cA + ci
                    sl = slice(ci * M, (ci + 1) * M)
                    wcp = pp.tile([P, M], F32, tag="wcp", space="PSUM")
                    MMX(wcp[0:C, :], ec8[:, cc_ * C:(cc_ + 1) * C], wct8[:, :],
                        start=True, stop=True)
                    h2s = wp.tile([C, M], BF16, tag=f"h2s{ci}", name=f"h2s{ci}")
                    TT(out=h2s[:, :], in0=h2p[:, sl], in1=wcp[0:C, :], op=ALU.mult)
                    h2ss.append(h2s)
                for ci in range(2):
                    cc_ = cA + ci
                    MMX(pout[:, :], wpost[:, :], h2ss[ci][:, :], start=(cc_ == 0),
                        stop=(cc_ == 7))
            osb = wp.tile([45, M], F32, tag="osb")
            ACT(out=osb[:, :], in_=pout[:, :], func=AF.Identity, bias=postb[:, 0:1])
            nc.sync.dma_start(outD[:, cs:cs + M], osb[:, :])

    from concourse.library_overlay import lower_extended_insts
    lower_extended_insts(nc)
    if legalize:
        _legalize_waits(nc)
    return nc


def _wait_limit(ins):
    return 1


def _legalize_waits(nc):
    """The walrus codegen allows only one sync-wait command per normal
    instruction. Split excess waits onto dedicated same-engine NOPs
    inserted immediately before the instruction (program position is
    unchanged, so dependency semantics are identical)."""
    import bass_rust as _br
    nid = 0
    for fn in nc.m.functions:
        for bb in fn.blocks:
            il = bb.instructions
            i = 0
            while i < len(il):
                ins = il[i]
                si = ins.sync_info
                lim = _wait_limit(ins)
                if si is not None and si.on_wait and len(si.on_wait) > lim:
                    ow = list(si.on_wait)
                    keep, excess = ow[-lim:], ow[:-lim]
                    for w in excess:
                        nid += 1
                        nop = mybir.InstNoOp(name=f"legwait-{nid}", ins=[], outs=[])
                        nop.engine = ins.engine
                        nop.sync_info = _br.SyncInfo(on_wait=[w], on_update=[])
                        il.insert(i, nop)
                        i += 1
                    si.on_wait = keep
                i += 1


def make_in_maps(inputs, cst=None, nsh=NSH, ncore=NCORE):
    shared, percore = make_split_maps(inputs, cst, nsh, ncore)
    return [dict(shared, **pc) for pc in percore]


def make_split_maps(inputs, cst=None, nsh=NSH, ncore=NCORE):
    cst = cst or host_prep(inputs)
    qflat = cst["qflat"]
    wsh = nsh // 16
    wfull = cst["qw"].shape[1]
    lay, ncols = blob_layout(wfull)
    cblob = np.zeros((P, ncols), np.float32)
    for nm, (o, r, cc) in lay.items():
        arr = cst[nm]
        assert arr.shape == (r, cc), (nm, arr.shape, (r, cc))
        cblob[0:r, o:o + cc] = arr
    nb = sum(cc for _, _, cc in BLAY_BF)
    bblob = np.zeros((P, nb), BF)
    _o = 0
    for nm, r, cc in BLAY_BF:
        bblob[0:r, _o:_o + cc] = cst[nm]
        _o += cc
    shared = dict(table=np.ascontiguousarray(cst["table"]),
                  bblob=np.ascontiguousarray(bblob),
                  cblob=np.ascontiguousarray(cblob))
    percore = []
    for core in range(ncore):
        sl = qflat[:, core * nsh:(core + 1) * nsh]
        percore.append(dict(
            qc=np.ascontiguousarray(sl),
            qwsh=np.ascontiguousarray(
                sl.reshape(3, wsh, 16).transpose(0, 2, 1).reshape(48, wsh)),
        ))
    return shared, percore


_CACHE = {}


def get_runner(nc, ncore=NCORE):
    """Compile an SPMD runner: shared inputs replicated (one transfer),
    per-core inputs sharded along axis 0."""
    import concourse.bass2jax as b2j
    import jax
    from jax.sharding import Mesh, PartitionSpec, NamedSharding
    from jax.experimental.shard_map import shard_map

    b2j.install_neuronx_cc_hook()
    partition_name = nc.partition_id_tensor.name if nc.partition_id_tensor else None
    in_names, out_names, out_avals, zero_outs = [], [], [], []
    for alloc in nc.m.functions[0].allocations:
        if not isinstance(alloc, mybir.MemoryLocationSet):
            continue
        name = alloc.memorylocations[0].name
        if alloc.kind == "ExternalInput":
            if name != partition_name:
                in_names.append(name)
        elif alloc.kind == "ExternalOutput":
            out_names.append(name)
            shape = tuple(alloc.tensor_shape)
            dtype = mybir.dt.np(alloc.dtype)
            out_avals.append(jax.core.ShapedArray(shape, dtype))
            zero_outs.append(np.zeros(shape, dtype))
    n_params = len(in_names)
    all_names = in_names + out_names
    if partition_name is not None:
        all_names.append(partition_name)

    def _body(*args):
        operands = list(args)
        if partition_name is not None:
            operands.append(b2j.partition_id_tensor())
        outs = b2j._bass_exec_p.bind(
            *operands, out_avals=tuple(out_avals), in_names=tuple(all_names),
            out_names=tuple(out_names), lowering_input_output_aliases=(),
            sim_require_finite=True, sim_require_nnan=True, nc=nc)
        return tuple(outs)

    devices = jax.devices()[:ncore]
    mesh = Mesh(np.asarray(devices), ("core",))

    def runner(shared, percore):
        specs, arrs = [], []
        for nm in in_names:
            if nm in shared:
                specs.append(PartitionSpec())
                arrs.append(shared[nm])
            else:
                specs.append(PartitionSpec("core"))
                arrs.append(np.concatenate([pc[nm] for pc in percore], axis=0))
        for z in zero_outs:
            specs.append(PartitionSpec("core"))
            arrs.append(np.concatenate([z] * ncore, axis=0))
        fn = jax.jit(shard_map(_body, mesh=mesh, in_specs=tuple(specs),
                               out_specs=(PartitionSpec("core"),) * len(out_names),
                               check_rep=False), keep_unused=True)
        dev = [jax.device_put(a, NamedSharding(mesh, s)) for a, s in zip(arrs, specs)]
        outs = fn(*dev)
        jax.block_until_ready(outs)
        return {nm: np.asarray(o) for nm, o in zip(out_names, outs)}, (fn, dev)

    return runner


def kernel(**inputs):
    cst = host_prep(inputs)
    if "nc" not in _CACHE:
        _CACHE["nc"] = build(geo=cst["geo"])
        _CACHE["runner"] = get_runner(_CACHE["nc"])
    shared, percore = make_split_maps(inputs, cst)
    outmap, _ = _CACHE["runner"](shared, percore)
    full_cat = outmap["out"]                     # (8*45, NSH)
    full = np.concatenate([full_cat[c * 45:(c + 1) * 45] for c in range(NCORE)], axis=1)
    return full.reshape(1, 45, QE, QE, QE).astype(np.float32)



# revision 22
# speedup vs baseline: 1.5304x; 1.2443x over previous
"""Trainium2 Bass kernel for nn_ContRepDecoder (8-core SPMD, data-parallel over query points).

Strategy:
- Host builds a (32768, 1024) bf16 table: row v = concat over the 8 cube corners of the
  96 context features (zero-padded to 128) at voxel v+offset.
- Each core gathers its points' rows with dma_gather(transpose=True) -> feature-major
  [128, 8, M] tiles (corner-blocked), no on-chip transposes.
- Coordinate features (q, cc, rel_norm, fourier enc) are assembled into one f32
  "coordinate stack" SST [128, M] via tiny-K matmuls into PSUM + ACT Sin + range reduce.
- MLP: per corner 2 matmuls per L0 (gathered h + per-corner masked coord lhsT),
  silu on 2-corner psum pairs, residuals on DVE, trilinear combine via outer-product
  weight psum + pre-scaled post matmuls accumulating into one [45, M] psum.
- Offset-1 coordinate features are affine in offset-0 ones; the differences are folded
  into per-corner bias rows (host weights) + device-computed delta terms from the
  global rel minima (computed once per core over the FULL query set).
"""
import sys

for _p in ("/opt/trn_rl_repo",):
    if _p not in sys.path:
        sys.path.insert(0, _p)

import numpy as np
import ml_dtypes

import concourse.bass as bass
import concourse.mybir as mybir
import concourse.tile as tile

P = 128
C = 96
NFRQ = 8
GRID = 32
QE = 48
NPTS = QE ** 3
NCORE = 8
NSH = NPTS // NCORE
M = 512
EPS = 1e-7
PI = float(np.pi)
F32 = mybir.dt.float32
BF16 = mybir.dt.bfloat16
I16 = mybir.dt.int16
AF = mybir.ActivationFunctionType
ALU = mybir.AluOpType

# SST partition layout
NSIN, NCOS, NQ, NCC, NRN, NONE = 0, 48, 96, 99, 102, 105
BF = ml_dtypes.bfloat16


def corner_off(c):
    return (c >> 2 & 1, c >> 1 & 1, c & 1)


def blob_layout(wfull):
    """(name, rows, cols) entries packed along the free dim of one [128, W] f32 blob."""
    ents = [("hbm", C, 16), ("wdsm", 3, 16 * C),
            ("bl", C, 4), ("postb", 45, 1), ("emstk", 97, P),
            ("emd", 3, P), ("e105", 1, P),
            ("coeffidx", 48, 16),
            ("ident", P, P), ("qw", 48, wfull)]
    off, lay = 0, {}
    for nm, r, cc in ents:
        lay[nm] = (off, r, cc)
        off += cc
    return lay, off


BLAY_BF = [("wh0", P, C), ("wh1", C, C), ("w01", C, C), ("w02", C, C),
           ("w11", C, C), ("w12", C, C), ("wpost", C, 45),
           ("wcoord", P, 16 * C), ("sel6", 35, 24), ("ec8", 8, 8 * C)]
QOFF = 31.0


def _enc_row(d, part, k):
    # row of enc feature (dim d, part 0=sin 1=cos, freq k) within the 60-row coord block
    return 12 + d * 16 + part * 8 + k


def host_prep(inputs):
    cse = np.asarray(inputs["context_spatial_extent"], np.float32)
    origin = cse[0, :, 0, 0, 0].copy()
    vox = np.abs(cse[0, :, 1, 1, 1] - cse[0, :, 0, 0, 0])
    qvs = np.asarray(inputs["query_vox_size"], np.float32)[0]
    qflat = np.asarray(inputs["query_coord"], np.float32)[0].reshape(3, NPTS)

    ctx_t = np.moveaxis(np.asarray(inputs["context_v"], np.float32)[0], 0, -1)
    tmp = np.zeros((33, 33, 33, C), np.float32)
    tmp[:32, :32, :32] = ctx_t
    table = np.zeros((GRID ** 3, NCORE * P), np.float32)
    for c in range(8):
        i, j, k = corner_off(c)
        table[:, c * P:c * P + C] = tmp[i:i + 32, j:j + 32, k:k + 32, :].reshape(GRID ** 3, C)
    table = table.astype(BF)

    wfull = NPTS // 16
    qw = qflat.reshape(3, wfull, 16).transpose(0, 2, 1).reshape(48, wfull).copy()

    freqs = (5.0 ** (np.arange(NFRQ) / NFRQ)).astype(np.float32)
    ws = {k: np.asarray(v, np.float32) for k, v in inputs.items() if k.startswith(("blk", "post"))}

    def coord_lhsT(w0, b0):
        # sst coord-row basis (bf16-safe): v1 = vox*n - QOFF (exact odd ints),
        # v2 = cc0 - q  (in [-1,1]).  q = v1 - v2 + QOFF ; cc0 = v1 + QOFF.
        wc = w0[C:, :]
        out = np.zeros((8, P, C), np.float32)
        hostbias = np.zeros((8, C), np.float32)
        wds = np.zeros((8, 3, C), np.float32)
        for c in range(8):
            off = corner_off(c)
            hb = b0 + qvs @ wc[0:3]
            for d in range(3):
                o = off[d]
                for k in range(NFRQ):
                    out[c, NSIN + (2 * d + o) * 8 + k, :] = wc[_enc_row(d, 0, k)]
                    out[c, NCOS + (2 * d + o) * 8 + k, :] = wc[_enc_row(d, 1, k)]
                out[c, NQ + d, :] = wc[6 + d] + wc[3 + d]
                out[c, NCC + d, :] = -wc[6 + d]
                out[c, NRN + d, :] = wc[9 + d]
                wds[c, d, :] = o * wc[9 + d]
                hb = hb + (off[d] * float(vox[d])) * wc[3 + d] \
                        + QOFF * (wc[6 + d] + wc[3 + d])
            hostbias[c] = hb
        return out, hostbias, wds

    c0, hb0, wd0 = coord_lhsT(ws["blk0_w0"], ws["blk0_b0"])
    c1, hb1, wd1 = coord_lhsT(ws["blk1_w0"], ws["blk1_b0"])
    wcoord = np.concatenate([c0, c1], 0).transpose(1, 0, 2).reshape(P, 16 * C).copy()
    hbm = np.concatenate([hb0, hb1], 0).T.copy()
    wdsm = np.concatenate([wd0, wd1], 0).transpose(1, 0, 2).reshape(3, 16 * C).copy()

    wh0 = np.zeros((P, C), np.float32)
    wh0[:C] = ws["blk0_w0"][:C]

    # ang matmul lhsT stack [97, P] (32-aligned blocks for DVE partition-start
    # legality): rows 0-2 emr (rhs=rnq), 32-34 emq (rhs=qq), 64-66 emn
    # (rhs=n3q), row 96 = runtime offrow (rhs=ones).
    emr = np.zeros((3, P), np.float32)
    emq = np.zeros((3, P), np.float32)
    emn = np.zeros((3, P), np.float32)
    emd = np.zeros((3, P), np.float32)
    for d in range(3):
        for o in (0, 1):
            for k in range(NFRQ):
                w = 2 * PI * freqs[k]
                emr[d, NSIN + (2 * d + o) * 8 + k] = w
                emr[d, NCOS + (2 * d + o) * 8 + k] = w
                if o == 1:
                    emd[d, NSIN + (2 * d + 1) * 8 + k] = w
                    emd[d, NCOS + (2 * d + 1) * 8 + k] = w
        emr[d, NRN + d] = 1.0
        emq[d, NCC + d] = -1.0                 # v2 = vox*n - q
        emn[d, NCC + d] = float(vox[d])
        emn[d, NQ + d] = float(vox[d])         # v1 = vox*n - QOFF
    e105 = np.zeros((1, P), np.float32)
    e105[0, NONE] = 1.0
    e105[0, NCOS:NCOS + 48] = 0.5 * PI   # cos = sin(x + pi/2), folded into psum consts
    e105[0, NQ:NQ + 3] = -QOFF
    emstk = np.zeros((97, P), np.float32)
    emstk[0:3] = emr
    emstk[32:35] = emq
    emstk[64:67] = emn

    coeffidx = np.zeros((48, 16), np.float32)
    for ch, w in enumerate((GRID * GRID, GRID, 1)):
        for j in range(16):
            coeffidx[ch * 16 + j, j] = w
    ident = np.eye(P, dtype=np.float32)

    # trilinear factor selection [35, 24]: 3 blocks of [35,8]; rows 0:3 pick
    # from twq, rows 32:35 from omq (32-aligned stacked rhs).
    # F1 (x-offset i) uses t-row 2 (tx), F2 (j) row 1, F3 (k) row 0
    sel6 = np.zeros((35, 24), np.float32)
    for c in range(8):
        i, j, k = corner_off(c)
        sel6[2, 0 * 8 + c] = i       # F1 from tw
        sel6[34, 0 * 8 + c] = 1 - i  # F1 from om
        sel6[1, 1 * 8 + c] = j
        sel6[33, 1 * 8 + c] = 1 - j
        sel6[0, 2 * 8 + c] = k
        sel6[32, 2 * 8 + c] = 1 - k

    ec8 = np.zeros((8, 8 * C), np.float32)
    for c in range(8):
        ec8[c, c * C:(c + 1) * C] = 1.0

    # isotropic geometry -> immediate scalars
    assert np.allclose(vox, vox[0]) and np.allclose(origin, origin[0])
    geo = dict(orig=float(origin[0]), ivox=float(np.float32(1.0) / vox[0]),
               vox=float(vox[0]), clamp=float(-vox[0] / 2 + np.float32(EPS)),
               i15=float(np.float32(1.0) / (np.float32(1.5) * vox[0])))

    return dict(
        table=table, qw=qw, qflat=qflat,
        wcoord=wcoord.astype(BF), hbm=hbm, wdsm=wdsm,
        wh0=wh0.astype(BF), wh1=ws["blk1_w0"][:C].astype(BF),
        w01=ws["blk0_w1"].astype(BF), w02=ws["blk0_w2"].astype(BF),
        w11=ws["blk1_w1"].astype(BF), w12=ws["blk1_w2"].astype(BF),
        wpost=ws["post_w"].astype(BF),
        bl=np.stack([ws["blk0_b1"], ws["blk0_b2"], ws["blk1_b1"], ws["blk1_b2"]], 1).copy(),
        postb=ws["post_b"][:, None].copy(),
        emstk=emstk, emd=emd, e105=e105,
        coeffidx=coeffidx, ident=ident, sel6=sel6.astype(BF), ec8=ec8.astype(BF), geo=geo,
        origin=origin, vox=vox,
    )


def build(nsh=NSH, nfull=NPTS, act_silu=True, geo=None, legalize=True):
    geo = geo or dict(orig=0.0, ivox=0.5, vox=2.0, clamp=float(-1.0 + np.float32(EPS)), i15=float(np.float32(1.0) / np.float32(3.0)))
    G_OR, G_IV, G_VX, G_CL, G_I15 = geo["orig"], geo["ivox"], geo["vox"], geo["clamp"], geo["i15"]
    MAGIC = float(2.0 ** 23)
    INV2PI = float(np.float32(1.0) / np.float32(2 * PI))
    NEG2PI = float(-2 * PI)
    wfull = nfull // 16
    wsh = nsh // 16
    nt = nsh // M
    act_fn = AF.Silu if act_silu else AF.Tanh

    lay, ncols = blob_layout(wfull)
    nbcols = sum(cc for _, _, cc in BLAY_BF)
    nc = bass.Bass()
    dp = nc.declare_dram_parameter
    tableD = dp("table", [GRID ** 3, NCORE * P], BF16, isOutput=False)
    qcD = dp("qc", [3, nsh], F32, isOutput=False)
    qwshD = dp("qwsh", [48, wsh], F32, isOutput=False)
    cblobD = dp("cblob", [P, ncols], F32, isOutput=False)
    bblobD = dp("bblob", [P, nbcols], BF16, isOutput=False)
    outD = dp("out", [45, nsh], F32, isOutput=True)

    TS, TT = nc.vector.tensor_scalar, nc.vector.tensor_tensor
    MMX = nc.tensor.matmul
    ACT = nc.scalar.activation

    from contextlib import ExitStack
    with tile.TileContext(nc) as tc, ExitStack() as es:
        cp = es.enter_context(tc.tile_pool(name="const", bufs=1))
        p1 = es.enter_context(tc.tile_pool(name="ph1", bufs=1))
        wp = es.enter_context(tc.tile_pool(name="work", bufs=2))
        pp = es.enter_context(tc.tile_pool(name="ps", bufs=1, space="PSUM"))

        from concourse import library_config
        nc.gpsimd.load_library(library_config.mlp)

        cbt = cp.tile([P, ncols], F32, tag="cblob")
        nc.sync.dma_start(cbt[:, :], cblobD[:, :])
        bb = cp.tile([P, nbcols], BF16, tag="bblob")
        nc.sync.dma_start(bb[:, :], bblobD[:, :])

        def cv(nm):
            o, r, cc = lay[nm]
            return cbt[0:r, o:o + cc]
        bf_off = {}
        _o = 0
        for nm, r, cc in BLAY_BF:
            bf_off[nm] = (_o, r, cc)
            _o += cc

        def bv(nm):
            o, r, cc = bf_off[nm]
            return bb[0:r, o:o + cc]
        wcoord = bv("wcoord")
        hbm = cv("hbm")
        wds = cv("wdsm")
        bl = cv("bl")
        postb = cv("postb")
        emstk = cv("emstk")
        emd = cv("emd")
        e105 = cv("e105")
        coeff = cv("coeffidx")
        sel6 = bv("sel6")
        ec8 = bv("ec8")
        ident = cv("ident")
        wh0 = bv("wh0")
        wh1 = bv("wh1")
        wl = {k: bv(k) for k in ("w01", "w02", "w11", "w12")}
        wpost = bv("wpost")
        ones1 = cp.tile([1, M], F32, tag="ones1")
        nc.vector.memset(ones1[:, :], 1.0)
        ones11 = cp.tile([1, 1], F32, tag="ones11")
        nc.vector.memset(ones11[:, :], 1.0)
        ones96 = cp.tile([1, C], F32, tag="ones96")
        nc.vector.memset(ones96[:, :], 1.0)
        zc = cp.tile([C, 1], F32, tag="zc")
        nc.vector.memset(zc[:, :], 0.0)

        def nearest_chunk(srctile, c0, wc, tagp="p1"):
            """Compute clipped nearest (f32) for a chunk of a resident wrapped tile."""
            qch = srctile[:, c0:c0 + wc]
            y = p1.tile([48, CH1], F32, tag=tagp + "a", name="nck_y")
            TS(out=y[:, :wc], in0=qch, scalar1=G_OR, scalar2=G_IV,
               op0=ALU.subtract, op1=ALU.mult)
            n = p1.tile([48, CH1], F32, tag=tagp + "c", name="nck_n")
            TS(out=n[:, :wc], in0=y[:, :wc], scalar1=MAGIC, scalar2=MAGIC,
               op0=ALU.add, op1=ALU.subtract)
            TS(out=n[:, :wc], in0=n[:, :wc], scalar1=0.0, scalar2=float(GRID - 2),
               op0=ALU.max, op1=ALU.min)
            return qch, n

        CH1 = 1728
        qwsh = cp.tile([48, wsh], F32, tag="qwsh")
        nc.sync.dma_start(qwsh[:, :], qwshD[:, :])

        # ===== PHASE 2: gather indices for this core's shard ====
        idx128 = cp.tile([P, wsh], I16, tag="idx128")
        ch = 432
        for h in range(0, wsh, ch):
            wc = min(ch, wsh - h)
            _, nsw = nearest_chunk(qwsh, h, wc, tagp="p2")
            ix_ps = pp.tile([128, M], F32, tag="scr", space="PSUM")
            MMX(ix_ps[0:16, 0:wc], coeff[:, :], nsw[:, :wc], start=True, stop=True)
            nc.vector.tensor_copy(idx128[0:16, h:h + wc], ix_ps[0:16, 0:wc])
        for rep in (16, 32, 64):
            nc.gpsimd.dma_start(out=idx128[rep:2 * rep, :], in_=idx128[0:rep, :])

        # ===== PHASE 1: global rel minima over the FULL query set (wrapped layout) ====
        CH1 = 1728
        nch1 = (wfull + CH1 - 1) // CH1
        qwall = cv("qw")
        qwsh = cp.tile([48, wsh], F32, tag="qwsh")
        nc.sync.dma_start(qwsh[:, :], qwshD[:, :])


        m2ch = p1.tile([48, 2 * nch1], F32, tag="m2ch")
        for chi in range(nch1):
            c0 = chi * CH1
            wc = min(CH1, wfull - c0)
            qch, n = nearest_chunk(qwall, c0, wc)
            cw = p1.tile([48, CH1], F32, tag="p1a")
            TS(out=cw[:, :wc], in0=n[:, :wc], scalar1=G_VX, scalar2=G_OR,
               op0=ALU.mult, op1=ALU.add)
            dw = p1.tile([48, CH1], F32, tag="p1b")
            TT(out=dw[:, :wc], in0=cw[:, :wc], in1=qch, op=ALU.subtract)
            r0 = p1.tile([48, CH1], F32, tag="p1a")
            TS(out=r0[:, :wc], in0=dw[:, :wc], scalar1=G_CL, scalar2=None, op0=ALU.max)
            nc.vector.tensor_reduce(out=m2ch[:, chi:chi + 1], in_=r0[:, :wc],
                                    axis=mybir.AxisListType.X, op=ALU.min)
            r1 = p1.tile([48, CH1], F32, tag="p1c")
            TS(out=r1[:, :wc], in0=dw[:, :wc], scalar1=G_VX, scalar2=G_CL,
               op0=ALU.add, op1=ALU.max)
            nc.vector.tensor_reduce(out=m2ch[:, nch1 + chi:nch1 + chi + 1], in_=r1[:, :wc],
                                    axis=mybir.AxisListType.X, op=ALU.min)
        m2 = p1.tile([48, 2], F32, tag="m2")
        nc.vector.tensor_reduce(out=m2[:, 0:1], in_=m2ch[:, 0:nch1],
                                axis=mybir.AxisListType.X, op=ALU.min)
        nc.vector.tensor_reduce(out=m2[:, 1:2], in_=m2ch[:, nch1:2 * nch1],
                                axis=mybir.AxisListType.X, op=ALU.min)
        mt_ps = pp.tile([128, M], F32, tag="scr", space="PSUM")
        nc.tensor.transpose(out=mt_ps[0:2, 0:48], in_=m2[:, :], identity=ident[0:48, 0:48])
        mts = p1.tile([2, 48], F32, tag="mts")
        ACT(out=mts[:, :], in_=mt_ps[0:2, 0:48], func=AF.Copy)
        mn6 = p1.tile([2, 3], F32, tag="mn6")
        for d in range(3):
            nc.vector.tensor_reduce(out=mn6[:, d:d + 1], in_=mts[:, 16 * d:16 * (d + 1)],
                                    axis=mybir.AxisListType.X, op=ALU.min)
        mnt_ps = pp.tile([128, M], F32, tag="scr", space="PSUM")
        nc.tensor.transpose(out=mnt_ps[0:3, 0:2], in_=mn6[:, :], identity=ident[0:2, 0:2])
        mns = p1.tile([3, 2], F32, tag="mns")
        ACT(out=mns[:, :], in_=mnt_ps[0:3, 0:2], func=AF.Copy)
        # delta_d = (vox + min0 - min1) / (1.5 vox); negm0c = -min0/(1.5 vox)
        dcol = p1.tile([3, 1], F32, tag="dcol")
        TT(out=dcol[:, :], in0=mns[:, 0:1], in1=mns[:, 1:2], op=ALU.subtract)
        TS(out=dcol[:, :], in0=dcol[:, :], scalar1=G_VX, scalar2=G_I15,
           op0=ALU.add, op1=ALU.mult)
        negm0c = p1.tile([3, 1], F32, tag="negm0c")
        TS(out=negm0c[:, :], in0=mns[:, 0:1], scalar1=G_I15, scalar2=-1.0,
           op0=ALU.mult, op1=ALU.mult)
        # offrow [1,128]: delta angle offsets + ones row marker
        or_ps = pp.tile([128, M], F32, tag="scr", space="PSUM")
        MMX(or_ps[0:P, 0:1], emd[:, :], dcol[:, :], start=True, stop=False)
        MMX(or_ps[0:P, 0:1], e105[:, :], ones11[:, :], start=False, stop=True)
        orS = p1.tile([P, 1], F32, tag="orS")
        ACT(out=orS[:, :], in_=or_ps[0:P, 0:1], func=AF.Copy)
        ort_ps = pp.tile([128, M], F32, tag="scr", space="PSUM")
        nc.tensor.transpose(out=ort_ps[0:1, 0:P], in_=orS[:, :], identity=ident[:, :])
        offrow = p1.tile([1, P], F32, tag="offrow")
        ACT(out=offrow[:, :], in_=ort_ps[0:1, 0:P], func=AF.Copy)
        nc.gpsimd.dma_start(out=emstk[96:97, :], in_=offrow[:, :])
        # per-(block,corner) bias rows -> wcoord row NONE via transpose + DMAs
        db_ps = pp.tile([128, M], F32, tag="scr", space="PSUM")
        for cb in range(16):
            MMX(db_ps[0:C, cb:cb + 1], wds[:, cb * C:(cb + 1) * C], dcol[:, :],
                start=True, stop=True)
        biasT = p1.tile([C, 16], F32, tag="biasT")
        TT(out=biasT[:, :], in0=db_ps[0:C, 0:16], in1=hbm[:, :], op=ALU.add)
        bt_ps = pp.tile([128, M], F32, tag="scr", space="PSUM")
        nc.tensor.transpose(out=bt_ps[0:16, 0:C], in_=biasT[:, :], identity=ident[0:C, 0:C])
        biasTT = p1.tile([16, C], BF16, tag="biasTT")
        ACT(out=biasTT[:, :], in_=bt_ps[0:16, 0:C], func=AF.Copy)
        for cb in range(16):
            nc.gpsimd.dma_start(out=wcoord[NONE:NONE + 1, cb * C:(cb + 1) * C],
                                in_=biasTT[cb:cb + 1, :])

        # ===== PHASE 3: main loop ====
        qcv = qcD[:, :]
        for t in range(nt):
            cs = t * M
            GT = wp.tile([P, NCORE, M], BF16, tag="gt")
            nc.gpsimd.dma_gather(
                out_ap=GT[:, :, :], in_ap=tableD[:, :],
                idxs_ap=idx128[:, t * (M // 16):(t + 1) * (M // 16)],
                num_idxs=M, num_idxs_reg=M, elem_size=NCORE * P,
                transpose=True, queue_num=0,
            )
            # stacked ang rhs [97, M] (32-aligned blocks): rows 0:3 rnq,
            # 32:35 qq, 64:67 n3q, 96 ones; gap rows zeroed (lhsT is 0 there,
            # but garbage could be inf/nan).
            stk = wp.tile([97, M], F32, tag="stk")
            if t < 2:
                nc.vector.memset(stk[:, :], 0.0)
                nc.vector.memset(stk[96:97, :], 1.0)
            nc.sync.dma_start(stk[32:35, :], qcv[:, cs:cs + M])
            qq3 = wp.tile([3, M], F32, tag="qq3")
            nc.sync.dma_start(qq3[:, :], qcv[:, cs:cs + M])
            qq = qq3[:, :]
            yq = wp.tile([3, M], F32, tag="qsA")
            TS(out=yq[:, :], in0=qq, scalar1=G_OR,
               scalar2=G_IV, op0=ALU.subtract, op1=ALU.mult)
            n3q = stk[64:67, :]
            TS(out=n3q, in0=yq[:, :], scalar1=MAGIC, scalar2=MAGIC,
               op0=ALU.add, op1=ALU.subtract)
            TS(out=n3q, in0=n3q, scalar1=0.0, scalar2=float(GRID - 2),
               op0=ALU.max, op1=ALU.min)
            ccq = wp.tile([3, M], F32, tag="qsA")
            TS(out=ccq[:, :], in0=n3q, scalar1=G_VX,
               scalar2=G_OR, op0=ALU.mult, op1=ALU.add)
            r0q = wp.tile([3, M], F32, tag="qsB")
            TT(out=r0q[:, :], in0=ccq[:, :], in1=qq, op=ALU.subtract)
            TS(out=r0q[:, :], in0=r0q[:, :], scalar1=G_CL,
               scalar2=None, op0=ALU.max)
            TS(out=stk[0:3, :], in0=r0q[:, :], scalar1=G_I15,
               scalar2=negm0c[:, 0:1], op0=ALU.mult, op1=ALU.add)
            # stacked trilinear rhs [35, M]: rows 0:3 twq, 32:35 omq
            tom = wp.tile([35, M], BF16, tag="twq")
            if t < 2:
                nc.vector.memset(tom[:, :], 0.0)
            twq = tom[0:3, :]
            TS(out=twq, in0=r0q[:, :], scalar1=-0.5,
               scalar2=-(1.0 - EPS) / 2, op0=ALU.add, op1=ALU.max)
            TS(out=twq, in0=twq, scalar1=(1.0 - EPS) / 2,
               scalar2=0.5, op0=ALU.min, op1=ALU.add)
            TS(out=tom[32:35, :], in0=twq, scalar1=-1.0, scalar2=1.0,
               op0=ALU.mult, op1=ALU.add)

            ang = pp.tile([P, M], F32, tag="ang", space="PSUM")
            MMX(ang[:, :], emstk[:, :], stk[:, :], start=True, stop=True)
            sst = wp.tile([P, M], BF16, tag="sst")
            rr = wp.tile([P, M], F32, tag="rr")
            red = wp.tile([P, M], F32, tag="red")
            TS(out=rr[0:C, :], in0=ang[0:C, :], scalar1=INV2PI, scalar2=MAGIC,
               op0=ALU.mult, op1=ALU.add)
            TS(out=rr[0:C, :], in0=rr[0:C, :], scalar1=MAGIC, scalar2=NEG2PI,
               op0=ALU.subtract, op1=ALU.mult)
            TT(out=red[0:C, :], in0=ang[0:C, :], in1=rr[0:C, :], op=ALU.add)
            ACT(out=sst[0:C, :], in_=red[0:C, :], func=AF.Sin, bias=0.0)
            ACT(out=sst[C:P, :], in_=ang[C:P, :], func=AF.Copy)

            # trilinear factors: F1/F2/F3 [8, M] psums via selection matmuls, then products
            fps = []
            for fi, tg in enumerate(("wcp", "pout", "ang")):
                f_ps = pp.tile([P, M], F32, tag=tg, space="PSUM",
                               name=f"f{fi}_ps")
                MMX(f_ps[0:8, :], sel6[:, fi * 8:(fi + 1) * 8], tom[:, :],
                    start=True, stop=True)
                fps.append(f_ps)
            f2s = wp.tile([8, M], F32, tag="f2s")
            nc.vector.tensor_copy(f2s[:, :], fps[1][0:8, :])
            w12t = wp.tile([8, M], F32, tag="w12t")
            TT(out=w12t[:, :], in0=fps[0][0:8, :], in1=f2s[:, :], op=ALU.mult)
            wct8 = wp.tile([8, M], BF16, tag="wct8")
            TT(out=wct8[:, :], in0=w12t[:, :], in1=fps[2][0:8, :], op=ALU.mult)

            pout = pp.tile([45, M], F32, tag="pout", space="PSUM")
            # two cpair chains interleaved stage-by-stage; sub0 uses psum
            # tags lp0/lp1, sub1 reuses scr/ang (free after the prologue).
            for grp in range(2):
                st = [{} for _ in range(2)]
                tgs = [("lp0", "lp1"), ("lp1", "lp0")]
                def S_lp0(s):
                    cA = (2 * grp + s) * 2
                    lp0 = pp.tile([C, 2 * M], F32, tag=tgs[s][0], space="PSUM")
                    for ci in range(2):
                        MMX(lp0[:, ci * M:(ci + 1) * M], wh0[:, :], GT[:, cA + ci, :],
                            start=True, stop=False)
                    for ci in range(2):
                        cc_ = cA + ci
                        MMX(lp0[:, ci * M:(ci + 1) * M],
                            wcoord[:, cc_ * C:(cc_ + 1) * C], sst[:, :],
                            start=False, stop=True)
                    st[s]["lp0"] = lp0
                def S_t1(s):
                    t1p = wp.tile([C, 2 * M], BF16, tag=f"t1p{s}", bufs=2)
                    ACT(out=t1p[:, :], in_=st[s]["lp0"][:, :], func=act_fn, bias=zc[:, 0:1])
                    st[s]["t1p"] = t1p
                def S_lp1(s):
                    lp1 = pp.tile([C, 2 * M], F32, tag=tgs[s][1], space="PSUM")
                    for ci in range(2):
                        sl = slice(ci * M, (ci + 1) * M)
                        MMX(lp1[:, sl], wl["w01"][:, :], st[s]["t1p"][:, sl], start=True, stop=True)
                    st[s]["lp1"] = lp1
                def S_t2(s):
                    t2p = wp.tile([C, 2 * M], BF16, tag=f"t2p{s}", bufs=2)
                    ACT(out=t2p[:, :], in_=st[s]["lp1"][:, :], func=act_fn, bias=bl[:, 0:1])
                    st[s]["t2p"] = t2p
                def S_lp2(s):
                    lp2 = pp.tile([C, 2 * M], F32, tag=tgs[s][0], space="PSUM")
                    for ci in range(2):
                        sl = slice(ci * M, (ci + 1) * M)
                        MMX(lp2[:, sl], wl["w02"][:, :], st[s]["t2p"][:, sl], start=True, stop=True)
                    st[s]["lp2"] = lp2
                def S_t3h1(s):
                    cA = (2 * grp + s) * 2
                    t3p = wp.tile([C, 2 * M], BF16, tag=f"t3p{s}", bufs=2)
                    ACT(out=t3p[:, :], in_=st[s]["lp2"][:, :], func=act_fn, bias=bl[:, 1:2])
                    h1p = wp.tile([C, 2 * M], BF16, tag=f"h1p{s}")
                    TT(out=h1p[:, :], in0=GT[0:C, cA:cA + 2, :], in1=t3p[:, :], op=ALU.add)
                    st[s]["h1p"] = h1p
                def S_lp3(s):
                    cA = (2 * grp + s) * 2
                    lp3 = pp.tile([C, 2 * M], F32, tag=tgs[s][1], space="PSUM")
                    for ci in range(2):
                        sl = slice(ci * M, (ci + 1) * M)
                        MMX(lp3[:, sl], wh1[:, :], st[s]["h1p"][:, sl], start=True, stop=False)
                    for ci in range(2):
                        cc_ = cA + ci
                        MMX(lp3[:, ci * M:(ci + 1) * M],
                            wcoord[:, (8 + cc_) * C:(9 + cc_) * C], sst[:, :],
                            start=False, stop=True)
                    st[s]["lp3"] = lp3
                def S_u1(s):
                    u1p = wp.tile([C, 2 * M], BF16, tag=f"t1p{s}", bufs=2, name="u1p")
                    ACT(out=u1p[:, :], in_=st[s]["lp3"][:, :], func=act_fn, bias=zc[:, 0:1])
                    st[s]["u1p"] = u1p
                def S_lp4(s):
                    lp4 = pp.tile([C, 2 * M], F32, tag=tgs[s][0], space="PSUM")
                    for ci in range(2):
                        sl = slice(ci * M, (ci + 1) * M)
                        MMX(lp4[:, sl], wl["w11"][:, :], st[s]["u1p"][:, sl], start=True, stop=True)
                    st[s]["lp4"] = lp4
                def S_u2(s):
                    u2p = wp.tile([C, 2 * M], BF16, tag=f"t2p{s}", bufs=2, name="u2p")
                    ACT(out=u2p[:, :], in_=st[s]["lp4"][:, :], func=act_fn, bias=bl[:, 2:3])
                    st[s]["u2p"] = u2p
                def S_lp5(s):
                    lp5 = pp.tile([C, 2 * M], F32, tag=tgs[s][1], space="PSUM")
                    for ci in range(2):
                        sl = slice(ci * M, (ci + 1) * M)
                        MMX(lp5[:, sl], wl["w12"][:, :], st[s]["u2p"][:, sl], start=True, stop=True)
                    st[s]["lp5"] = lp5
                def S_tail(s):
                    cA = (2 * grp + s) * 2
                    u3p = wp.tile([C, 2 * M], BF16, tag=f"t3p{s}", bufs=2, name="u3p")
                    ACT(out=u3p[:, :], in_=st[s]["lp5"][:, :], func=act_fn, bias=bl[:, 3:4])
                    h2p = wp.tile([C, 2 * M], BF16, tag=f"h2p{s}")
                    TT(out=h2p[:, :], in0=st[s]["h1p"][:, :], in1=u3p[:, :], op=ALU.add)
                    h2ss = []
                    for ci in range(2):
                        cc_ = cA + ci
                        sl = slice(ci * M, (ci + 1) * M)
                        wcp = pp.tile([P, M], F32, tag="wcp", space="PSUM")
                        MMX(wcp[0:C, :], ec8[:, cc_ * C:(cc_ + 1) * C], wct8[:, :],
                            start=True, stop=True)
                        h2s = wp.tile([C, M], BF16, tag=f"h2s{s}{ci}", name=f"h2s{s}{ci}")
                        TT(out=h2s[:, :], in0=h2p[:, sl], in1=wcp[0:C, :], op=ALU.mult)
                        h2ss.append(h2s)
                    for ci in range(2):
                        cc_ = cA + ci
                        MMX(pout[:, :], wpost[:, :], h2ss[ci][:, :], start=(cc_ == 0),
                            stop=(cc_ == 7))
                for stage in (S_lp0, S_t1, S_lp1, S_t2, S_lp2, S_t3h1, S_lp3,
                              S_u1, S_lp4, S_u2, S_lp5, S_tail):
                    for s in range(2):
                        stage(s)
            osb = wp.tile([45, M], F32, tag="osb")
            ACT(out=osb[:, :], in_=pout[:, :], func=AF.Identity, bias=postb[:, 0:1])
            nc.sync.dma_start(outD[:, cs:cs + M], osb[:, :])

    from concourse.library_overlay import lower_extended_insts
    lower_extended_insts(nc)
    if legalize:
        _legalize_waits(nc)
    return nc


def _wait_limit(ins):
    return 1


def _legalize_waits(nc):
    """The walrus codegen allows only one sync-wait command per normal
    instruction. Split excess waits onto dedicated same-engine NOPs
    inserted immediately before the instruction (program position is
    unchanged, so dependency semantics are identical)."""
    import bass_rust as _br
    nid = 0
    for fn in nc.m.functions:
        for bb in fn.blocks:
            il = bb.instructions
            i = 0
            while i < len(il):
                ins = il[i]
                si = ins.sync_info
                lim = _wait_limit(ins)
                if si is not None and si.on_wait and len(si.on_wait) > lim:
                    ow = list(si.on_wait)
                    keep, excess = ow[-lim:], ow[:-lim]
                    for w in excess:
                        nid += 1
                        nop = mybir.InstNoOp(name=f"legwait-{nid}", ins=[], outs=[])
                        nop.engine = ins.engine
                        nop.sync_info = _br.SyncInfo(on_wait=[w], on_update=[])
                        il.insert(i, nop)
                        i += 1
                    si.on_wait = keep
                i += 1


def make_in_maps(inputs, cst=None, nsh=NSH, ncore=NCORE):
    shared, percore = make_split_maps(inputs, cst, nsh, ncore)
    return [dict(shared, **pc) for pc in percore]


def make_split_maps(inputs, cst=None, nsh=NSH, ncore=NCORE):
    cst = cst or host_prep(inputs)
    qflat = cst["qflat"]
    wsh = nsh // 16
    wfull = cst["qw"].shape[1]
    lay, ncols = blob_layout(wfull)
    cblob = np.zeros((P, ncols), np.float32)
    for nm, (o, r, cc) in lay.items():
        arr = cst[nm]
        assert arr.shape == (r, cc), (nm, arr.shape, (r, cc))
        cblob[0:r, o:o + cc] = arr
    nb = sum(cc for _, _, cc in BLAY_BF)
    bblob = np.zeros((P, nb), BF)
    _o = 0
    for nm, r, cc in BLAY_BF:
        bblob[0:r, _o:_o + cc] = cst[nm]
        _o += cc
    shared = dict(table=np.ascontiguousarray(cst["table"]),
                  bblob=np.ascontiguousarray(bblob),
                  cblob=np.ascontiguousarray(cblob))
    percore = []
    for core in range(ncore):
        sl = qflat[:, core * nsh:(core + 1) * nsh]
        percore.append(dict(
            qc=np.ascontiguousarray(sl),
            qwsh=np.ascontiguousarray(
                sl.reshape(3, wsh, 16).transpose(0, 2, 1).reshape(48, wsh)),
        ))
    return shared, percore


_CACHE = {}


def get_runner(nc, ncore=NCORE):
    """Compile an SPMD runner: shared inputs replicated (one transfer),
    per-core inputs sharded along axis 0."""
    import concourse.bass2jax as b2j
    import jax
    from jax.sharding import Mesh, PartitionSpec, NamedSharding
    from jax.experimental.shard_map import shard_map

    b2j.install_neuronx_cc_hook()
    partition_name = nc.partition_id_tensor.name if nc.partition_id_tensor else None
    in_names, out_names, out_avals, zero_outs = [], [], [], []
    for alloc in nc.m.functions[0].allocations:
        if not isinstance(alloc, mybir.MemoryLocationSet):
            continue
        name = alloc.memorylocations[0].name
        if alloc.kind == "ExternalInput":
            if name != partition_name:
                in_names.append(name)
        elif alloc.kind == "ExternalOutput":
            out_names.append(name)
            shape = tuple(alloc.tensor_shape)
            dtype = mybir.dt.np(alloc.dtype)
            out_avals.append(jax.core.ShapedArray(shape, dtype))
            zero_outs.append(np.zeros(shape, dtype))
    n_params = len(in_names)
    all_names = in_names + out_names
    if partition_name is not None:
        all_names.append(partition_name)

    def _body(*args):
        operands = list(args)
        if partition_name is not None:
            operands.append(b2j.partition_id_tensor())
        outs = b2j._bass_exec_p.bind(
            *operands, out_avals=tuple(out_avals), in_names=tuple(all_names),
            out_names=tuple(out_names), lowering_input_output_aliases=(),
            sim_require_finite=True, sim_require_nnan=True, nc=nc)
        return tuple(outs)

    devices = jax.devices()[:ncore]
    mesh = Mesh(np.asarray(devices), ("core",))

    def runner(shared, percore):
        specs, arrs = [], []
        for nm in in_names:
            if nm in shared:
                specs.append(PartitionSpec())
                arrs.append(shared[nm])
            else:
                specs.append(PartitionSpec("core"))
                arrs.append(np.concatenate([pc[nm] for pc in percore], axis=0))
        for z in zero_outs:
            specs.append(PartitionSpec("core"))
            arrs.append(np.concatenate([z] * ncore, axis=0))
        fn = jax.jit(shard_map(_body, mesh=mesh, in_specs=tuple(specs),
                               out_specs=(PartitionSpec("core"),) * len(out_names),
                               check_rep=False), keep_unused=True)
        dev = [jax.device_put(a, NamedSharding(mesh, s)) for a, s in zip(arrs, specs)]
        outs = fn(*dev)
        jax.block_until_ready(outs)
        return {nm: np.asarray(o) for nm, o in zip(out_names, outs)}, (fn, dev)

    return runner


def kernel(**inputs):
    cst = host_prep(inputs)
    if "nc" not in _CACHE:
        _CACHE["nc"] = build(geo=cst["geo"])
        _CACHE["runner"] = get_runner(_CACHE["nc"])
    shared, percore = make_split_maps(inputs, cst)
    outmap, _ = _CACHE["runner"](shared, percore)
    full_cat = outmap["out"]                     # (8*45, NSH)
    full = np.concatenate([full_cat[c * 45:(c + 1) * 45] for c in range(NCORE)], axis=1)
    return full.reshape(1, 45, QE, QE, QE).astype(np.float32)

